# revision 27
# baseline (speedup 1.0000x reference)
"""Head-sharded (tensor-parallel) causal attention block for 8 NeuronCores.

Model: B=2, S=2048, D=1024, H=16 heads (HD=64). Each core owns 2 heads
(128 features) of the QKV projections and attention, computes a partial
output projection (o_shard @ ow_shard), and the host sums the 8 partials
and adds the output bias.

Layout (single PSUM scope, chunk-interleaved pipeline):
  - Q/K projections run in fp8e4 DoubleRow mode (2 k-tiles of 128 per
    pass -> 4 passes instead of 8, 2x PE throughput). Host supplies
    x*16 and qw*256/kw*256 in fp8; the PSUM result carries a 4096x
    scale that the fused bias-add (tensor_scalar mult+add) removes.
    V projection stays fp16 (v errors hit the output directly).
  - Projection and attention work interleave at j-tile granularity via
    generators: ready projection matmuls sit between potentially
    exp-stalled QK ops in the PE queue, so ScalarE's exp stream (the
    attention-phase pacer) overlaps projection matmuls.
  - V is projected into vT [feat, seq] (fp16), then moved to the PV
    lhsT layout V_aug[t, feat] via DMA XBAR transposes (no PE/PSUM).
  - V_aug columns 64:128 hold 1.0: the PV matmul emits the softmax
    denominator replicated on partitions 64:128, so normalization is a
    plain reciprocal + multiply (no partition broadcast).
  - The attention j-loop is software-pipelined: QK+exp for j are
    emitted one stage ahead of PV(j-1), so the in-order PE queue never
    parks on a PV waiting for its exp -- filler matmuls slot in behind
    the next QK instead. Causal mask: post-exp multiply of the diagonal
    128-block by a static 0/1 mask (DVE), emitted a full stage before
    PV consumes it. Fully-masked columns are skipped via col0 = 128*k.
  - Output projection partial[sq,1024] = oT.T @ owT in 512-wide halves
    through the projection PSUM pool; PSUM->SBUF copies ride DVE (ACT
    must stay free for the exp stream; GPSIMD cannot touch PSUM). The
    chunk's normalization (reciprocal of the PV-emitted denominator +
    multiply) is interleaved per 256-col half with the output
    projection that consumes it. The last chunks' projections are
    deferred past the final attention chunk.
  - PSUM budget (8 banks): proj/outproj 2x1, scores 2x2, PV accum 2x1.

Measured on the 8-core axon pod: ~199 us/body (baseline 219 us), rel
err 1.58e-2 vs the fp32 reference (gate 2e-2; the error is dominated
by the deliberate fp8 Q/K projections, measured identical in numpy
emulation).
"""

import numpy as np

import concourse.bass as bass
import concourse.mybir as mybir
import concourse.tile as tile
from concourse import bacc
from concourse.bass import ts
from concourse.bass_utils import run_bass_kernel_spmd

B, S, D, H = 2, 2048, 1024, 16
HD = D // H            # 64 head dim
NCORES = 8
FPC = D // NCORES      # 128 features per core
HPC = FPC // HD        # 2 heads per core
P = 128
SQ_CHUNK = 512         # query chunk (matmul free dim)
NSQ = S // SQ_CHUNK    # 4
NTB = S // P           # 16 t-blocks
DBLK = D // P          # 8 contraction blocks for fp16 projections
DBLK2 = DBLK // 2      # 4 DoubleRow passes for fp8 projections

F32 = mybir.dt.float32
MM_DT = mybir.dt.float16
F8 = mybir.dt.float8e4
X8_SCALE = 16.0        # x -> fp8 scale
W8_SCALE = 256.0       # qw/kw -> fp8 scale
DESCALE = 1.0 / (X8_SCALE * W8_SCALE)

_module_cache = {}


def _build_module(repeat=1, hwloop=False, unroll=1):
    nc = bacc.Bacc("TRN2", target_bir_lowering=False, debug=False)

    xT_d = nc.dram_tensor("xT", [B, D, S], MM_DT, kind="ExternalInput").ap()
    xT8_d = nc.dram_tensor("xT8", [B, D, S], F8, kind="ExternalInput").ap()
    qwT8_d = nc.dram_tensor("qwT8", [D, FPC], F8, kind="ExternalInput").ap()
    kwT8_d = nc.dram_tensor("kwT8", [D, FPC], F8, kind="ExternalInput").ap()
    vwT_d = nc.dram_tensor("vwT", [D, FPC], MM_DT, kind="ExternalInput").ap()
    qb_d = nc.dram_tensor("qb", [FPC, 1], F32, kind="ExternalInput").ap()
    kb_d = nc.dram_tensor("kb", [FPC, 1], F32, kind="ExternalInput").ap()
    vb_d = nc.dram_tensor("vb", [FPC, 1], F32, kind="ExternalInput").ap()
    owT_d = nc.dram_tensor("owT", [FPC, D], MM_DT, kind="ExternalInput").ap()
    out_d = nc.dram_tensor("out", [B, S, D], MM_DT, kind="ExternalOutput").ap()

    # [B, D, S] with D split into 8 blocks of 128 partitions
    xT_r = xT_d.rearrange("b (o p) s -> b p o s", p=P)
    # fp8 x in DoubleRow pair layout: d = 256*o2 + 128*two + p
    xT8_r = xT8_d.rearrange("b (o2 two p) s -> b p o2 two s", two=2, p=P)
    # [B, S, D] with S split into 128-row blocks (partition-first)
    out_r = out_d.rearrange("b (o p) d -> b p o d", p=P)

    with tile.TileContext(nc) as tc:
        with (
            tc.tile_pool(name="singles", bufs=1) as singles,
            tc.tile_pool(name="xin", bufs=3) as xin,
            tc.tile_pool(name="x8in", bufs=3) as x8in,
            tc.tile_pool(name="ptile", bufs=5) as ptile,
            tc.tile_pool(name="small", bufs=4) as small,
            tc.tile_pool(name="outsb", bufs=3) as outsb,
        ):
            # --- constants / persistent tensors (loaded once) ---
            qw8_sb = singles.tile([P, DBLK2, 2, FPC], F8, tag="qw8")
            kw8_sb = singles.tile([P, DBLK2, 2, FPC], F8, tag="kw8")
            vwT_sb = singles.tile([P, DBLK, FPC], MM_DT, tag="vw")
            nc.sync.dma_start(
                out=qw8_sb,
                in_=qwT8_d.rearrange("(o2 two p) m -> p o2 two m", two=2, p=P))
            nc.sync.dma_start(
                out=kw8_sb,
                in_=kwT8_d.rearrange("(o2 two p) m -> p o2 two m", two=2, p=P))
            nc.sync.dma_start(out=vwT_sb, in_=vwT_d.rearrange("(o p) m -> p o m", p=P))
            qb_sb = singles.tile([FPC, 1], F32, tag="qb")
            kb_sb = singles.tile([FPC, 1], F32, tag="kb")
            vb_sb = singles.tile([FPC, 1], F32, tag="vb")
            nc.sync.dma_start(out=qb_sb, in_=qb_d)
            nc.sync.dma_start(out=kb_sb, in_=kb_d)
            nc.sync.dma_start(out=vb_sb, in_=vb_d)
            owT_sb = singles.tile([FPC, D], MM_DT, tag="ow")
            nc.sync.dma_start(out=owT_sb, in_=owT_d)

            qT_sb = singles.tile([P, B, S], MM_DT, tag="qT")
            kT_sb = singles.tile([P, B, S], MM_DT, tag="kT")
            vT_sb = singles.tile([P, B, S], MM_DT, tag="vT")
            oT_sb = singles.tile([P, B, S], MM_DT, tag="oT")
            # V_aug[t, b, h, tblk, 0:64] = v features (fp16, written
            # directly by the DMA XBAR transposes); [.., 64:128] = 1.0 ->
            # the PV matmul emits the softmax denominator replicated on
            # partitions 64:128
            v_aug = singles.tile([P, B, HPC, NTB, P], MM_DT, tag="vaug")
            ones_sb = singles.tile([P, 1], F32, tag="ones")
            nc.vector.memset(ones_sb, 1.0)
            nc.vector.tensor_copy(
                out=v_aug[:, :, :, :, HD:P],
                in_=ones_sb[:, 0][:, None, None, None, None].to_broadcast(
                    [P, B, HPC, NTB, HD]),
            )
            # static causal 0/1 mask (fp16) for the post-exp multiply:
            # m128[t, c] = (c >= t)
            m128 = singles.tile([P, P], MM_DT, tag="m128")
            nc.gpsimd.memset(m128, 1.0)
            nc.gpsimd.affine_select(
                out=m128, in_=m128, compare_op=mybir.AluOpType.is_ge,
                fill=0.0, base=0, pattern=[[1, P]], channel_multiplier=-1,
            )

            # PSUM pools span all repetitions so consecutive bodies
            # pipeline through slot rotation instead of draining at each
            # body boundary
            with (
                tc.tile_pool(name="pps", bufs=2, space="PSUM") as pps,
                tc.tile_pool(name="mpsum", bufs=2, space="PSUM") as mpsum,
                tc.tile_pool(name="opsum", bufs=2, space="PSUM") as opsum,
            ):
                # ------ repetitions (>1 only for HW timing calibration) --
                if hwloop and repeat > 1:
                    with tc.For_i(0, repeat) as _i:
                        for _u in range(unroll):
                            _emit_body(nc, tc, locals())
                else:
                    for _rep in range(repeat):
                        _emit_body(nc, tc, locals())

    return nc


def _emit_body(nc, tc, env):
    g = type("G", (), env)
    singles, xin, x8in, ptile, small, outsb = (
        g.singles, g.xin, g.x8in, g.ptile, g.small, g.outsb)
    qw8_sb, kw8_sb, vwT_sb = g.qw8_sb, g.kw8_sb, g.vwT_sb
    qb_sb, kb_sb, vb_sb, owT_sb = g.qb_sb, g.kb_sb, g.vb_sb, g.owT_sb
    qT_sb, kT_sb, vT_sb, oT_sb, v_aug = g.qT_sb, g.kT_sb, g.vT_sb, g.oT_sb, g.v_aug
    m128 = g.m128
    xT_r, xT8_r, out_r = g.xT_r, g.xT8_r, g.out_r
    pps, mpsum, opsum = g.pps, g.mpsum, g.opsum

    if True:
        def ph1_chunk(b, cn):
            # generator: yields between matmul groups so the emitter can
            # interleave projection work into the attention j-loop.
            xt8 = x8in.tile([P, DBLK2, 2, SQ_CHUNK], F8, tag="xt8",
                            name=f"xt8_{b}{cn}")
            for g2 in range(2):
                nc.sync.dma_start(
                    out=xt8[:, 2 * g2:2 * g2 + 2],
                    in_=xT8_r[b, :, 2 * g2:2 * g2 + 2, :, ts(cn, SQ_CHUNK)])
            xt = xin.tile([P, DBLK, SQ_CHUNK], MM_DT, tag="xt",
                          name=f"xt{b}{cn}")
            for qd in range(4):
                nc.sync.dma_start(
                    out=xt[:, 2 * qd:2 * qd + 2, :],
                    in_=xT_r[b, :, 2 * qd:2 * qd + 2, ts(cn, SQ_CHUNK)])
            # Q/K in fp8 DoubleRow (4 passes of 2x128 contraction)
            for w8_sb, bias_sb, dst in (
                (qw8_sb, qb_sb, qT_sb),
                (kw8_sb, kb_sb, kT_sb),
            ):
                ps = pps.tile([P, SQ_CHUNK], F32, tag="pps",
                              name=f"prj{b}{cn}{id(dst)%97}")
                for o2 in range(DBLK2):
                    nc.tensor.matmul(
                        ps,
                        lhsT=w8_sb[:, o2],
                        rhs=xt8[:, o2],
                        perf_mode=mybir.MatmulPerfMode.DoubleRow,
                        start=(o2 == 0),
                        stop=(o2 == DBLK2 - 1),
                    )
                    if o2 == 1:
                        yield
                # fused descale (1/4096) + bias add, fp32 PSUM -> fp16 SBUF
                nc.vector.tensor_scalar(
                    out=dst[:, b, ts(cn, SQ_CHUNK)], in0=ps,
                    scalar1=DESCALE, scalar2=bias_sb,
                    op0=mybir.AluOpType.mult, op1=mybir.AluOpType.add,
                )
                yield
            # V projection in fp16 (8 passes)
            ps = pps.tile([P, SQ_CHUNK], F32, tag="pps",
                          name=f"prjv{b}{cn}")
            for o in range(DBLK):
                nc.tensor.matmul(
                    ps,
                    lhsT=vwT_sb[:, o, :],
                    rhs=xt[:, o, :],
                    start=(o == 0),
                    stop=(o == DBLK - 1),
                )
                if o % 4 == 3:
                    yield
            nc.vector.tensor_scalar_add(
                out=vT_sb[:, b, ts(cn, SQ_CHUNK)], in0=ps, scalar1=vb_sb,
            )
            yield
            # move V of this chunk into PV-lhsT layout via DMA XBAR
            # transpose: [64 feat, 512 seq] -> [128 t x 4 blocks, 64 feat]
            for h in range(HPC):
                hs = h * HD
                nc.sync.dma_start(
                    out=v_aug[:, b, h, 4 * cn:4 * cn + 4, 0:HD],
                    in_=vT_sb[hs:hs + HD, b, ts(cn, SQ_CHUNK)],
                    transpose=True,
                )

        def att_chunk(b, i, po_h):
            # software-pipelined j-loop: QK+exp for j run one stage ahead
            # of PV(j-1), so the PE's in-order queue never parks on a PV
            # that waits for its exp — filler matmuls (projections, output
            # projections) slot in behind QK(j+1) instead.
            jmax = 4 * i + 3
            pend = None
            for j in range(jmax + 2):
                if j <= jmax:
                    # columns < 128k of diagonal blocks are fully masked;
                    # skip them in QK, exp and PV
                    k = j - 4 * i
                    col0 = P * k if k > 0 else 0
                    ps = mpsum.tile([P, HPC, SQ_CHUNK], F32, tag="ps",
                                    name=f"ps{b}{i}{j}")
                    # two heads' QK in adjacent PE row-tiles
                    for h in range(HPC):
                        hs = h * HD
                        nc.tensor.matmul(
                            ps[:, h, col0:],
                            lhsT=kT_sb[hs:hs + HD, b, ts(j, P)],
                            rhs=qT_sb[hs:hs + HD, b,
                                      i * SQ_CHUNK + col0:(i + 1) * SQ_CHUNK],
                            start=True,
                            stop=True,
                        )
                    pt = ptile.tile([P, HPC, SQ_CHUNK], MM_DT, tag="pt",
                                    name=f"pt{b}{i}{j}")
                    nc.scalar.activation(
                        out=pt[:, :, col0:], in_=ps[:, :, col0:],
                        func=mybir.ActivationFunctionType.Exp,
                        scale=0.125,
                    )
                    if k >= 0:
                        # causal zero-fill post-exp; lands a full stage
                        # before PV consumes the diagonal block
                        nc.vector.tensor_tensor(
                            out=pt[:, :, col0:col0 + P],
                            in0=pt[:, :, col0:col0 + P],
                            in1=m128[:, None, :].to_broadcast([P, HPC, P]),
                            op=mybir.AluOpType.mult,
                        )
                    cur = (j, col0, pt)
                else:
                    cur = None
                if pend is not None:
                    pj, pcol0, ppt = pend
                    for h in range(HPC):
                        nc.tensor.matmul(
                            po_h[h][:, pcol0:],
                            lhsT=v_aug[:, b, h, pj, :],
                            rhs=ppt[:, h, pcol0:],
                            start=(pj == 0),
                            stop=(pj == jmax),
                            skip_group_check=True,
                        )
                pend = cur
                yield

        def flush_norm(b, i, po_h):
            for h in range(HPC):
                hs = h * HD
                rb = small.tile([HD, SQ_CHUNK], F32, tag="rb",
                                name=f"rb{b}{i}{h}")
                nc.vector.reciprocal(out=rb, in_=po_h[h][HD:P, :])
                # split by 256-col halves so the first output-projection
                # s-blocks can start before the full chunk is normalized
                for q in range(2):
                    qs = q * (SQ_CHUNK // 2)
                    nc.vector.tensor_mul(
                        out=oT_sb[hs:hs + HD, b,
                                  i * SQ_CHUNK + qs:
                                  i * SQ_CHUNK + qs + SQ_CHUNK // 2],
                        in0=po_h[h][0:HD, qs:qs + SQ_CHUNK // 2],
                        in1=rb[:, qs:qs + SQ_CHUNK // 2],
                    )

        def flush_proj(b, i, norm_po=None, rbs=None, tail=False):
            # generator; when norm_po is given, the normalization of each
            # 256-col half is emitted just before the output-projection
            # matmuls that consume it, shrinking the serial chunk-boundary
            # section
            if rbs is None:
                rbs = [None, None]
            for half in range(2):
                if norm_po is not None:
                    qs = half * (SQ_CHUNK // 2)
                    for h in range(HPC):
                        hs = h * HD
                        if half == 0:
                            rb = small.tile([HD, SQ_CHUNK], F32, tag="rb",
                                            name=f"rb{b}{i}{h}")
                            rbs[h] = rb
                            nc.vector.reciprocal(
                                out=rb, in_=norm_po[h][HD:P, :])
                        nc.vector.tensor_mul(
                            out=oT_sb[hs:hs + HD, b,
                                      i * SQ_CHUNK + qs:
                                      i * SQ_CHUNK + qs + SQ_CHUNK // 2],
                            in0=norm_po[h][0:HD, qs:qs + SQ_CHUNK // 2],
                            in1=rbs[h][:, qs:qs + SQ_CHUNK // 2],
                        )
                ot = outsb.tile([P, 2, D], MM_DT, tag="ot",
                                name=f"ot{b}_{i}_{half}")
                for si in range(2):
                    s = 4 * i + 2 * half + si
                    for cc in range(2):
                        # deferred (post-attention) flushes draw PSUM from
                        # the idle scores pool, so the next hwloop body's
                        # projections don't serialize behind the tail's
                        # DVE drain through the shared pps rotation
                        if tail:
                            pw = mpsum.tile([P, HPC, SQ_CHUNK], F32,
                                            tag="ps", name=f"tp{b}_{s}_{cc}")
                            pp = pw[:, 0, :]
                        else:
                            pp = pps.tile([P, SQ_CHUNK], F32, tag="pps",
                                          name=f"pp{b}_{s}_{cc}")
                        nc.tensor.matmul(
                            pp,
                            lhsT=oT_sb[:, b, ts(s, P)],
                            rhs=owT_sb[:, ts(cc, SQ_CHUNK)],
                            start=True,
                            stop=True,
                        )
                        # PSUM->SBUF moves all ride DVE: ACT must stay free
                        # for the exp stream (the attention-phase pacer) and
                        # GPSIMD cannot access PSUM on TRN2
                        nc.vector.tensor_copy(
                            out=ot[:, si, ts(cc, SQ_CHUNK)], in_=pp,
                        )
                    yield
                # rows [s0, s0+1] of this batch as [128, 2, D]
                s0 = 4 * i + 2 * half
                nc.sync.dma_start(out=out_r[b, :, s0:s0 + 2, :], in_=ot)

        # chunk-interleaved schedule: the projection generator for the
        # next chunk is drained round-robin with the attention j-loop of
        # the previous chunk, so ready projection matmuls sit between
        # potentially-stalling QK ops in the PE queue
        chunks = [(b, cn) for b in range(B) for cn in range(NSQ)]
        groups = []
        for n, (b, cn) in enumerate(chunks):
            prev = chunks[n - 1] if n > 0 else None
            groups.append(((b, cn), prev))
        groups.append((None, chunks[-1]))

        deferred = []
        for gi, (pitem, aitem) in enumerate(groups):
            gp = ph1_chunk(*pitem) if pitem is not None else None
            if aitem is not None:
                b, i = aitem
                po_h = [
                    opsum.tile([P, SQ_CHUNK], F32, tag="po",
                               name=f"po{b}_{i}_{h}")
                    for h in range(HPC)
                ]
                ga = att_chunk(b, i, po_h)
            else:
                ga = None
            while gp is not None or ga is not None:
                if gp is not None:
                    try:
                        next(gp)
                    except StopIteration:
                        gp = None
                if ga is not None:
                    try:
                        next(ga)
                    except StopIteration:
                        ga = None
            if aitem is not None:
                # defer the last chunks' output projections so they can
                # fill the exp-paced tail of the final attention chunk
                if gi >= len(groups) - 5:
                    flush_norm(b, i, po_h)
                    deferred.append((b, i))
                else:
                    for _ in flush_proj(b, i, norm_po=po_h,
                                        rbs=[None, None]):
                        pass
        for b, i in deferred:
            for _ in flush_proj(b, i, tail=True):
                pass


def get_module(repeat=1, hwloop=False, unroll=1):
    key = ("nc", repeat, hwloop, unroll)
    if key not in _module_cache:
        m = _build_module(repeat=repeat, hwloop=hwloop, unroll=unroll)
        m.compile()
        _module_cache[key] = m
    return _module_cache[key]


def make_in_maps(x, qw, qb, kw, kb, vw, vb, ow):
    mmdt = np.dtype(np.float16)
    f8dt = np.dtype(mybir.dt.np(mybir.dt.float8e4))
    xT = np.ascontiguousarray(x.transpose(0, 2, 1)).astype(mmdt)  # [B, D, S]
    xT8 = np.ascontiguousarray(
        x.transpose(0, 2, 1).astype(np.float32) * X8_SCALE).astype(f8dt)
    in_maps = []
    for c in range(NCORES):
        sl = slice(c * FPC, (c + 1) * FPC)
        m = {
            "xT": xT,
            "xT8": xT8,
            "qwT8": np.ascontiguousarray(
                qw[sl, :].T.astype(np.float32) * W8_SCALE).astype(f8dt),
            "kwT8": np.ascontiguousarray(
                kw[sl, :].T.astype(np.float32) * W8_SCALE).astype(f8dt),
            "vwT": np.ascontiguousarray(vw[sl, :].T).astype(mmdt),
            "qb": np.ascontiguousarray(qb[sl].reshape(FPC, 1)).astype(np.float32),
            "kb": np.ascontiguousarray(kb[sl].reshape(FPC, 1)).astype(np.float32),
            "vb": np.ascontiguousarray(vb[sl].reshape(FPC, 1)).astype(np.float32),
            "owT": np.ascontiguousarray(ow[:, sl].T).astype(mmdt),
        }
        in_maps.append(m)
    return in_maps


def kernel(x, qw, qb, kw, kb, vw, vb, ow, ob, _trace=False):
    x = np.asarray(x, dtype=np.float32)
    qw = np.asarray(qw, dtype=np.float32)
    qb = np.asarray(qb, dtype=np.float32)
    kw = np.asarray(kw, dtype=np.float32)
    kb = np.asarray(kb, dtype=np.float32)
    vw = np.asarray(vw, dtype=np.float32)
    vb = np.asarray(vb, dtype=np.float32)
    ow = np.asarray(ow, dtype=np.float32)
    ob = np.asarray(ob, dtype=np.float32)

    nc = get_module()
    in_maps = make_in_maps(x, qw, qb, kw, kb, vw, vb, ow)
    res = run_bass_kernel_spmd(
        nc, in_maps, core_ids=list(range(NCORES)), trace=_trace
    )
    acc = np.zeros((B, S, D), dtype=np.float64)
    for r in res.results:
        acc += r["out"].astype(np.float64)
    out = (acc + ob.astype(np.float64)).astype(np.float32)
    if _trace:
        kernel.last_results = res
    return out


# revision 28
# speedup vs baseline: 1.0544x; 1.0544x over previous
"""Head-sharded (tensor-parallel) causal attention block for 8 NeuronCores.

Model: B=2, S=2048, D=1024, H=16 heads (HD=64). Each core owns 2 heads
(128 features) of the QKV projections and attention, computes a partial
output projection (o_shard @ ow_shard), and the host sums the 8 partials
and adds the output bias.

Layout (single PSUM scope, chunk-interleaved pipeline):
  - Q/K projections run in fp8e4 DoubleRow mode (2 k-tiles of 128 per
    pass -> 4 passes instead of 8, 2x PE throughput). Host supplies
    x*16 and qw*256/kw*256 in fp8; the PSUM result carries a 4096x
    scale that the fused bias-add (tensor_scalar mult+add) removes.
    V projection stays fp16 (v errors hit the output directly).
  - Projection and attention work interleave at j-tile granularity via
    generators: ready projection matmuls sit between potentially
    exp-stalled QK ops in the PE queue, so ScalarE's exp stream (the
    attention-phase pacer) overlaps projection matmuls.
  - V is projected into vT [feat, seq] (fp16), then moved to the PV
    lhsT layout V_aug[t, feat] via DMA XBAR transposes (no PE/PSUM).
  - V_aug columns 64:128 hold 1.0: the PV matmul emits the softmax
    denominator replicated on partitions 64:128, so normalization is a
    plain reciprocal + multiply (no partition broadcast).
  - The attention j-loop is software-pipelined: QK+exp for j are
    emitted one stage ahead of PV(j-1), so the in-order PE queue never
    parks on a PV waiting for its exp -- filler matmuls slot in behind
    the next QK instead. Causal mask: post-exp multiply of the diagonal
    128-block by a static 0/1 mask (DVE), emitted a full stage before
    PV consumes it. Fully-masked columns are skipped via col0 = 128*k.
  - Output projection partial[sq,1024] = oT.T @ owT in 512-wide halves
    through the projection PSUM pool; PSUM->SBUF copies ride DVE (ACT
    must stay free for the exp stream; GPSIMD cannot touch PSUM). The
    chunk's normalization (reciprocal of the PV-emitted denominator +
    multiply) is interleaved per 256-col half with the output
    projection that consumes it. The last chunks' projections are
    deferred past the final attention chunk.
  - PSUM budget (8 banks): proj/outproj 2x1, scores 2x2, PV accum 2x1.

Measured on the 8-core axon pod: ~199 us/body (baseline 219 us), rel
err 1.58e-2 vs the fp32 reference (gate 2e-2; the error is dominated
by the deliberate fp8 Q/K projections, measured identical in numpy
emulation).
"""

import numpy as np

import concourse.bass as bass
import concourse.mybir as mybir
import concourse.tile as tile
from concourse import bacc
from concourse.bass import ts
from concourse.bass_utils import run_bass_kernel_spmd

B, S, D, H = 2, 2048, 1024, 16
HD = D // H            # 64 head dim
NCORES = 8
FPC = D // NCORES      # 128 features per core
HPC = FPC // HD        # 2 heads per core
P = 128
SQ_CHUNK = 512         # query chunk (matmul free dim)
NSQ = S // SQ_CHUNK    # 4
NTB = S // P           # 16 t-blocks
DBLK = D // P          # 8 contraction blocks for fp16 projections
DBLK2 = DBLK // 2      # 4 DoubleRow passes for fp8 projections

F32 = mybir.dt.float32
MM_DT = mybir.dt.float16
F8 = mybir.dt.float8e4
X8_SCALE = 16.0        # x -> fp8 scale
W8_SCALE = 256.0       # qw/kw -> fp8 scale
DESCALE = 1.0 / (X8_SCALE * W8_SCALE)

_module_cache = {}


def _build_module(repeat=1, hwloop=False, unroll=1):
    nc = bacc.Bacc("TRN2", target_bir_lowering=False, debug=False)

    xT_d = nc.dram_tensor("xT", [B, D, S], MM_DT, kind="ExternalInput").ap()
    xT8_d = nc.dram_tensor("xT8", [B, D, S], F8, kind="ExternalInput").ap()
    qwT8_d = nc.dram_tensor("qwT8", [D, FPC], F8, kind="ExternalInput").ap()
    kwT8_d = nc.dram_tensor("kwT8", [D, FPC], F8, kind="ExternalInput").ap()
    vwT_d = nc.dram_tensor("vwT", [D, FPC], MM_DT, kind="ExternalInput").ap()
    qb_d = nc.dram_tensor("qb", [FPC, 1], F32, kind="ExternalInput").ap()
    kb_d = nc.dram_tensor("kb", [FPC, 1], F32, kind="ExternalInput").ap()
    vb_d = nc.dram_tensor("vb", [FPC, 1], F32, kind="ExternalInput").ap()
    owT_d = nc.dram_tensor("owT", [FPC, D], MM_DT, kind="ExternalInput").ap()
    out_d = nc.dram_tensor("out", [B, S, D], MM_DT, kind="ExternalOutput").ap()

    # [B, D, S] with D split into 8 blocks of 128 partitions
    xT_r = xT_d.rearrange("b (o p) s -> b p o s", p=P)
    # fp8 x in DoubleRow pair layout: d = 256*o2 + 128*two + p
    xT8_r = xT8_d.rearrange("b (o2 two p) s -> b p o2 two s", two=2, p=P)
    # [B, S, D] with S split into 128-row blocks (partition-first)
    out_r = out_d.rearrange("b (o p) d -> b p o d", p=P)

    with tile.TileContext(nc) as tc:
        with (
            tc.tile_pool(name="singles", bufs=1) as singles,
            tc.tile_pool(name="xin", bufs=3) as xin,
            tc.tile_pool(name="x8in", bufs=3) as x8in,
            tc.tile_pool(name="ptile", bufs=5) as ptile,
            tc.tile_pool(name="small", bufs=4) as small,
            tc.tile_pool(name="outsb", bufs=3) as outsb,
        ):
            # --- constants / persistent tensors (loaded once) ---
            qw8_sb = singles.tile([P, DBLK2, 2, FPC], F8, tag="qw8")
            kw8_sb = singles.tile([P, DBLK2, 2, FPC], F8, tag="kw8")
            vwT_sb = singles.tile([P, DBLK, FPC], MM_DT, tag="vw")
            nc.sync.dma_start(
                out=qw8_sb,
                in_=qwT8_d.rearrange("(o2 two p) m -> p o2 two m", two=2, p=P))
            nc.sync.dma_start(
                out=kw8_sb,
                in_=kwT8_d.rearrange("(o2 two p) m -> p o2 two m", two=2, p=P))
            nc.sync.dma_start(out=vwT_sb, in_=vwT_d.rearrange("(o p) m -> p o m", p=P))
            qb_sb = singles.tile([FPC, 1], F32, tag="qb")
            kb_sb = singles.tile([FPC, 1], F32, tag="kb")
            vb_sb = singles.tile([FPC, 1], F32, tag="vb")
            nc.sync.dma_start(out=qb_sb, in_=qb_d)
            nc.sync.dma_start(out=kb_sb, in_=kb_d)
            nc.sync.dma_start(out=vb_sb, in_=vb_d)
            owT_sb = singles.tile([FPC, D], MM_DT, tag="ow")
            nc.sync.dma_start(out=owT_sb, in_=owT_d)

            qT_sb = singles.tile([P, B, S], MM_DT, tag="qT")
            kT_sb = singles.tile([P, B, S], MM_DT, tag="kT")
            vT_sb = singles.tile([P, B, S], MM_DT, tag="vT")
            oT_sb = singles.tile([P, B, S], MM_DT, tag="oT")
            # V_aug[t, b, h, tblk, 0:64] = v features (fp16, written
            # directly by the DMA XBAR transposes); [.., 64:128] = 1.0 ->
            # the PV matmul emits the softmax denominator replicated on
            # partitions 64:128
            v_aug = singles.tile([P, B, HPC, NTB, P], MM_DT, tag="vaug")
            ones_sb = singles.tile([P, 1], F32, tag="ones")
            nc.vector.memset(ones_sb, 1.0)
            nc.vector.tensor_copy(
                out=v_aug[:, :, :, :, HD:P],
                in_=ones_sb[:, 0][:, None, None, None, None].to_broadcast(
                    [P, B, HPC, NTB, HD]),
            )
            # static causal 0/1 mask (fp16) for the post-exp multiply:
            # m128[t, c] = (c >= t)
            m128 = singles.tile([P, P], MM_DT, tag="m128")
            nc.gpsimd.memset(m128, 1.0)
            nc.gpsimd.affine_select(
                out=m128, in_=m128, compare_op=mybir.AluOpType.is_ge,
                fill=0.0, base=0, pattern=[[1, P]], channel_multiplier=-1,
            )

            # PSUM pools span all repetitions so consecutive bodies
            # pipeline through slot rotation instead of draining at each
            # body boundary
            with (
                tc.tile_pool(name="pps", bufs=2, space="PSUM") as pps,
                tc.tile_pool(name="mpsum", bufs=2, space="PSUM") as mpsum,
                tc.tile_pool(name="opsum", bufs=2, space="PSUM") as opsum,
            ):
                # ------ repetitions (>1 only for HW timing calibration) --
                if hwloop and repeat > 1:
                    with tc.For_i(0, repeat) as _i:
                        for _u in range(unroll):
                            _emit_body(nc, tc, locals())
                else:
                    for _rep in range(repeat):
                        _emit_body(nc, tc, locals())

    return nc


def _emit_body(nc, tc, env):
    g = type("G", (), env)
    singles, xin, x8in, ptile, small, outsb = (
        g.singles, g.xin, g.x8in, g.ptile, g.small, g.outsb)
    qw8_sb, kw8_sb, vwT_sb = g.qw8_sb, g.kw8_sb, g.vwT_sb
    qb_sb, kb_sb, vb_sb, owT_sb = g.qb_sb, g.kb_sb, g.vb_sb, g.owT_sb
    qT_sb, kT_sb, vT_sb, oT_sb, v_aug = g.qT_sb, g.kT_sb, g.vT_sb, g.oT_sb, g.v_aug
    m128 = g.m128
    xT_r, xT8_r, out_r = g.xT_r, g.xT8_r, g.out_r
    pps, mpsum, opsum = g.pps, g.mpsum, g.opsum

    if True:
        def ph1_chunk(b, cn):
            # generator: yields between matmul groups so the emitter can
            # interleave projection work into the attention j-loop.
            xt8 = x8in.tile([P, DBLK2, 2, SQ_CHUNK], F8, tag="xt8",
                            name=f"xt8_{b}{cn}")
            for g2 in range(2):
                nc.sync.dma_start(
                    out=xt8[:, 2 * g2:2 * g2 + 2],
                    in_=xT8_r[b, :, 2 * g2:2 * g2 + 2, :, ts(cn, SQ_CHUNK)])
            xt = xin.tile([P, DBLK, SQ_CHUNK], MM_DT, tag="xt",
                          name=f"xt{b}{cn}")
            for qd in range(4):
                nc.sync.dma_start(
                    out=xt[:, 2 * qd:2 * qd + 2, :],
                    in_=xT_r[b, :, 2 * qd:2 * qd + 2, ts(cn, SQ_CHUNK)])
            # Q/K in fp8 DoubleRow (4 passes of 2x128 contraction)
            for w8_sb, bias_sb, dst in (
                (qw8_sb, qb_sb, qT_sb),
                (kw8_sb, kb_sb, kT_sb),
            ):
                ps = pps.tile([P, SQ_CHUNK], F32, tag="pps",
                              name=f"prj{b}{cn}{id(dst)%97}")
                for o2 in range(DBLK2):
                    nc.tensor.matmul(
                        ps,
                        lhsT=w8_sb[:, o2],
                        rhs=xt8[:, o2],
                        perf_mode=mybir.MatmulPerfMode.DoubleRow,
                        start=(o2 == 0),
                        stop=(o2 == DBLK2 - 1),
                    )
                    if o2 == 1:
                        yield
                # fused descale (1/4096) + bias add, fp32 PSUM -> fp16 SBUF
                nc.vector.tensor_scalar(
                    out=dst[:, b, ts(cn, SQ_CHUNK)], in0=ps,
                    scalar1=DESCALE, scalar2=bias_sb,
                    op0=mybir.AluOpType.mult, op1=mybir.AluOpType.add,
                )
                yield
            # V projection in fp16 (8 passes)
            ps = pps.tile([P, SQ_CHUNK], F32, tag="pps",
                          name=f"prjv{b}{cn}")
            for o in range(DBLK):
                nc.tensor.matmul(
                    ps,
                    lhsT=vwT_sb[:, o, :],
                    rhs=xt[:, o, :],
                    start=(o == 0),
                    stop=(o == DBLK - 1),
                )
                if o % 4 == 3:
                    yield
            nc.vector.tensor_scalar_add(
                out=vT_sb[:, b, ts(cn, SQ_CHUNK)], in0=ps, scalar1=vb_sb,
            )
            yield
            # move V of this chunk into PV-lhsT layout via DMA XBAR
            # transpose: [64 feat, 512 seq] -> [128 t x 4 blocks, 64 feat]
            for h in range(HPC):
                hs = h * HD
                nc.sync.dma_start(
                    out=v_aug[:, b, h, 4 * cn:4 * cn + 4, 0:HD],
                    in_=vT_sb[hs:hs + HD, b, ts(cn, SQ_CHUNK)],
                    transpose=True,
                )

        def att_chunk(b, i, po_h):
            # software-pipelined j-loop: QK+exp for j run one stage ahead
            # of PV(j-1), so the PE's in-order queue never parks on a PV
            # that waits for its exp — filler matmuls (projections, output
            # projections) slot in behind QK(j+1) instead.
            jmax = 4 * i + 3
            pend = None
            for j in range(jmax + 2):
                if j <= jmax:
                    # columns < 128k of diagonal blocks are fully masked;
                    # skip them in QK, exp and PV
                    k = j - 4 * i
                    col0 = P * k if k > 0 else 0
                    ps = mpsum.tile([P, HPC, SQ_CHUNK], F32, tag="ps",
                                    name=f"ps{b}{i}{j}")
                    # two heads' QK in adjacent PE row-tiles
                    for h in range(HPC):
                        hs = h * HD
                        nc.tensor.matmul(
                            ps[:, h, col0:],
                            lhsT=kT_sb[hs:hs + HD, b, ts(j, P)],
                            rhs=qT_sb[hs:hs + HD, b,
                                      i * SQ_CHUNK + col0:(i + 1) * SQ_CHUNK],
                            start=True,
                            stop=True,
                        )
                    pt = ptile.tile([P, HPC, SQ_CHUNK], MM_DT, tag="pt",
                                    name=f"pt{b}{i}{j}")
                    nc.scalar.activation(
                        out=pt[:, :, col0:], in_=ps[:, :, col0:],
                        func=mybir.ActivationFunctionType.Exp,
                        scale=0.125,
                    )
                    if k >= 0:
                        # causal zero-fill post-exp; lands a full stage
                        # before PV consumes the diagonal block
                        nc.vector.tensor_tensor(
                            out=pt[:, :, col0:col0 + P],
                            in0=pt[:, :, col0:col0 + P],
                            in1=m128[:, None, :].to_broadcast([P, HPC, P]),
                            op=mybir.AluOpType.mult,
                        )
                    cur = (j, col0, pt)
                else:
                    cur = None
                if pend is not None:
                    pj, pcol0, ppt = pend
                    for h in range(HPC):
                        nc.tensor.matmul(
                            po_h[h][:, pcol0:],
                            lhsT=v_aug[:, b, h, pj, :],
                            rhs=ppt[:, h, pcol0:],
                            start=(pj == 0),
                            stop=(pj == jmax),
                            skip_group_check=True,
                        )
                pend = cur
                yield

        def flush_norm(b, i, po_h):
            for h in range(HPC):
                hs = h * HD
                rb = small.tile([HD, SQ_CHUNK], F32, tag="rb",
                                name=f"rb{b}{i}{h}")
                nc.vector.reciprocal(out=rb, in_=po_h[h][HD:P, :])
                # split by 256-col halves so the first output-projection
                # s-blocks can start before the full chunk is normalized
                for q in range(2):
                    qs = q * (SQ_CHUNK // 2)
                    nc.vector.tensor_mul(
                        out=oT_sb[hs:hs + HD, b,
                                  i * SQ_CHUNK + qs:
                                  i * SQ_CHUNK + qs + SQ_CHUNK // 2],
                        in0=po_h[h][0:HD, qs:qs + SQ_CHUNK // 2],
                        in1=rb[:, qs:qs + SQ_CHUNK // 2],
                    )

        def flush_proj(b, i, norm_po=None, rbs=None, tail=False):
            # generator; when norm_po is given, the normalization of each
            # 256-col half is emitted just before the output-projection
            # matmuls that consume it, shrinking the serial chunk-boundary
            # section
            if rbs is None:
                rbs = [None, None]
            for half in range(2):
                if norm_po is not None:
                    qs = half * (SQ_CHUNK // 2)
                    for h in range(HPC):
                        hs = h * HD
                        if half == 0:
                            rb = small.tile([HD, SQ_CHUNK], F32, tag="rb",
                                            name=f"rb{b}{i}{h}")
                            rbs[h] = rb
                            nc.vector.reciprocal(
                                out=rb, in_=norm_po[h][HD:P, :])
                        nc.vector.tensor_mul(
                            out=oT_sb[hs:hs + HD, b,
                                      i * SQ_CHUNK + qs:
                                      i * SQ_CHUNK + qs + SQ_CHUNK // 2],
                            in0=norm_po[h][0:HD, qs:qs + SQ_CHUNK // 2],
                            in1=rbs[h][:, qs:qs + SQ_CHUNK // 2],
                        )
                ot = outsb.tile([P, 2, D], MM_DT, tag="ot",
                                name=f"ot{b}_{i}_{half}")
                for si in range(2):
                    s = 4 * i + 2 * half + si
                    for cc in range(2):
                        # post-attention (tail) flushes split both the PSUM
                        # pool (pps/mpsum) and the drain engine (DVE/ACT):
                        # the serial tail drain halves, and the next hwloop
                        # body's projections (gated on pps via DVE) and
                        # attention (gated on mpsum via ACT) both restart
                        # ~13us earlier. Inline flushes keep DVE-only (ACT
                        # is busy with the exp stream there).
                        odd = tail and cc == 1
                        if odd:
                            pw = mpsum.tile([P, HPC, SQ_CHUNK], F32,
                                            tag="ps", name=f"tp{b}_{s}_{cc}")
                            pp = pw[:, 0, :]
                        else:
                            pp = pps.tile([P, SQ_CHUNK], F32, tag="pps",
                                          name=f"pp{b}_{s}_{cc}")
                        nc.tensor.matmul(
                            pp,
                            lhsT=oT_sb[:, b, ts(s, P)],
                            rhs=owT_sb[:, ts(cc, SQ_CHUNK)],
                            start=True,
                            stop=True,
                        )
                        if odd:
                            nc.scalar.copy(
                                out=ot[:, si, ts(cc, SQ_CHUNK)], in_=pp,
                            )
                        else:
                            nc.vector.tensor_copy(
                                out=ot[:, si, ts(cc, SQ_CHUNK)], in_=pp,
                            )
                    yield
                # rows [s0, s0+1] of this batch as [128, 2, D]
                s0 = 4 * i + 2 * half
                nc.sync.dma_start(out=out_r[b, :, s0:s0 + 2, :], in_=ot)

        # chunk-interleaved schedule: the projection generator for the
        # next chunk is drained round-robin with the attention j-loop of
        # the previous chunk, so ready projection matmuls sit between
        # potentially-stalling QK ops in the PE queue
        chunks = [(b, cn) for b in range(B) for cn in range(NSQ)]
        groups = []
        for n, (b, cn) in enumerate(chunks):
            prev = chunks[n - 1] if n > 0 else None
            groups.append(((b, cn), prev))
        groups.append((None, chunks[-1]))

        deferred = []
        for gi, (pitem, aitem) in enumerate(groups):
            gp = ph1_chunk(*pitem) if pitem is not None else None
            if aitem is not None:
                b, i = aitem
                po_h = [
                    opsum.tile([P, SQ_CHUNK], F32, tag="po",
                               name=f"po{b}_{i}_{h}")
                    for h in range(HPC)
                ]
                ga = att_chunk(b, i, po_h)
            else:
                ga = None
            while gp is not None or ga is not None:
                if gp is not None:
                    try:
                        next(gp)
                    except StopIteration:
                        gp = None
                if ga is not None:
                    try:
                        next(ga)
                    except StopIteration:
                        ga = None
            if aitem is not None:
                # defer the last chunks' output projections so they can
                # fill the exp-paced tail of the final attention chunk
                if gi >= len(groups) - 5:
                    flush_norm(b, i, po_h)
                    deferred.append((b, i))
                else:
                    for _ in flush_proj(b, i, norm_po=po_h,
                                        rbs=[None, None]):
                        pass
        for b, i in deferred:
            for _ in flush_proj(b, i, tail=True):
                pass


def get_module(repeat=1, hwloop=False, unroll=1):
    key = ("nc", repeat, hwloop, unroll)
    if key not in _module_cache:
        m = _build_module(repeat=repeat, hwloop=hwloop, unroll=unroll)
        m.compile()
        _module_cache[key] = m
    return _module_cache[key]


def make_in_maps(x, qw, qb, kw, kb, vw, vb, ow):
    mmdt = np.dtype(np.float16)
    f8dt = np.dtype(mybir.dt.np(mybir.dt.float8e4))
    xT = np.ascontiguousarray(x.transpose(0, 2, 1)).astype(mmdt)  # [B, D, S]
    xT8 = np.ascontiguousarray(
        x.transpose(0, 2, 1).astype(np.float32) * X8_SCALE).astype(f8dt)
    in_maps = []
    for c in range(NCORES):
        sl = slice(c * FPC, (c + 1) * FPC)
        m = {
            "xT": xT,
            "xT8": xT8,
            "qwT8": np.ascontiguousarray(
                qw[sl, :].T.astype(np.float32) * W8_SCALE).astype(f8dt),
            "kwT8": np.ascontiguousarray(
                kw[sl, :].T.astype(np.float32) * W8_SCALE).astype(f8dt),
            "vwT": np.ascontiguousarray(vw[sl, :].T).astype(mmdt),
            "qb": np.ascontiguousarray(qb[sl].reshape(FPC, 1)).astype(np.float32),
            "kb": np.ascontiguousarray(kb[sl].reshape(FPC, 1)).astype(np.float32),
            "vb": np.ascontiguousarray(vb[sl].reshape(FPC, 1)).astype(np.float32),
            "owT": np.ascontiguousarray(ow[:, sl].T).astype(mmdt),
        }
        in_maps.append(m)
    return in_maps


def kernel(x, qw, qb, kw, kb, vw, vb, ow, ob, _trace=False):
    x = np.asarray(x, dtype=np.float32)
    qw = np.asarray(qw, dtype=np.float32)
    qb = np.asarray(qb, dtype=np.float32)
    kw = np.asarray(kw, dtype=np.float32)
    kb = np.asarray(kb, dtype=np.float32)
    vw = np.asarray(vw, dtype=np.float32)
    vb = np.asarray(vb, dtype=np.float32)
    ow = np.asarray(ow, dtype=np.float32)
    ob = np.asarray(ob, dtype=np.float32)

    nc = get_module()
    in_maps = make_in_maps(x, qw, qb, kw, kb, vw, vb, ow)
    res = run_bass_kernel_spmd(
        nc, in_maps, core_ids=list(range(NCORES)), trace=_trace
    )
    acc = np.zeros((B, S, D), dtype=np.float64)
    for r in res.results:
        acc += r["out"].astype(np.float64)
    out = (acc + ob.astype(np.float64)).astype(np.float32)
    if _trace:
        kernel.last_results = res
    return out


# revision 29
# speedup vs baseline: 1.0586x; 1.0040x over previous
"""Head-sharded (tensor-parallel) causal attention block for 8 NeuronCores.

Model: B=2, S=2048, D=1024, H=16 heads (HD=64). Each core owns 2 heads
(128 features) of the QKV projections and attention, computes a partial
output projection (o_shard @ ow_shard), and the host sums the 8 partials
and adds the output bias.

Layout (single PSUM scope, chunk-interleaved pipeline):
  - Q/K projections run in fp8e4 DoubleRow mode (2 k-tiles of 128 per
    pass -> 4 passes instead of 8, 2x PE throughput). Host supplies
    x*16 and qw*256/kw*256 in fp8; the PSUM result carries a 4096x
    scale that the fused bias-add (tensor_scalar mult+add) removes.
    V projection stays fp16 (v errors hit the output directly).
  - Projection and attention work interleave at j-tile granularity via
    generators: ready projection matmuls sit between potentially
    exp-stalled QK ops in the PE queue, so ScalarE's exp stream (the
    attention-phase pacer) overlaps projection matmuls.
  - V is projected into vT [feat, seq] (fp16), then moved to the PV
    lhsT layout V_aug[t, feat] via DMA XBAR transposes (no PE/PSUM).
  - V_aug columns 64:128 hold 1.0: the PV matmul emits the softmax
    denominator replicated on partitions 64:128, so normalization is a
    plain reciprocal + multiply (no partition broadcast).
  - The attention j-loop is software-pipelined: QK+exp for j are
    emitted one stage ahead of PV(j-1), so the in-order PE queue never
    parks on a PV waiting for its exp -- filler matmuls slot in behind
    the next QK instead. Causal mask: post-exp multiply of the diagonal
    128-block by a static 0/1 mask (DVE), emitted a full stage before
    PV consumes it. Fully-masked columns are skipped via col0 = 128*k.
  - Output projection partial[sq,1024] = oT.T @ owT in 512-wide halves
    through the projection PSUM pool; PSUM->SBUF copies ride DVE (ACT
    must stay free for the exp stream; GPSIMD cannot touch PSUM). The
    chunk's normalization (reciprocal of the PV-emitted denominator +
    multiply) is interleaved per 256-col half with the output
    projection that consumes it. The last chunks' projections are
    deferred past the final attention chunk.
  - PSUM budget (8 banks): proj/outproj 2x1, scores 2x2, PV accum 2x1.

Measured on the 8-core axon pod: ~199 us/body (baseline 219 us), rel
err 1.58e-2 vs the fp32 reference (gate 2e-2; the error is dominated
by the deliberate fp8 Q/K projections, measured identical in numpy
emulation).
"""

import numpy as np

import concourse.bass as bass
import concourse.mybir as mybir
import concourse.tile as tile
from concourse import bacc
from concourse.bass import ts
from concourse.bass_utils import run_bass_kernel_spmd

B, S, D, H = 2, 2048, 1024, 16
HD = D // H            # 64 head dim
NCORES = 8
FPC = D // NCORES      # 128 features per core
HPC = FPC // HD        # 2 heads per core
P = 128
SQ_CHUNK = 512         # query chunk (matmul free dim)
NSQ = S // SQ_CHUNK    # 4
NTB = S // P           # 16 t-blocks
DBLK = D // P          # 8 contraction blocks for fp16 projections
DBLK2 = DBLK // 2      # 4 DoubleRow passes for fp8 projections

F32 = mybir.dt.float32
MM_DT = mybir.dt.float16
F8 = mybir.dt.float8e4
X8_SCALE = 16.0        # x -> fp8 scale
W8_SCALE = 256.0       # qw/kw -> fp8 scale
DESCALE = 1.0 / (X8_SCALE * W8_SCALE)

_module_cache = {}


def _build_module(repeat=1, hwloop=False, unroll=1):
    nc = bacc.Bacc("TRN2", target_bir_lowering=False, debug=False)

    xT_d = nc.dram_tensor("xT", [B, D, S], MM_DT, kind="ExternalInput").ap()
    xT8_d = nc.dram_tensor("xT8", [B, D, S], F8, kind="ExternalInput").ap()
    qwT8_d = nc.dram_tensor("qwT8", [D, FPC], F8, kind="ExternalInput").ap()
    kwT8_d = nc.dram_tensor("kwT8", [D, FPC], F8, kind="ExternalInput").ap()
    vwT_d = nc.dram_tensor("vwT", [D, FPC], MM_DT, kind="ExternalInput").ap()
    qb_d = nc.dram_tensor("qb", [FPC, 1], F32, kind="ExternalInput").ap()
    kb_d = nc.dram_tensor("kb", [FPC, 1], F32, kind="ExternalInput").ap()
    vb_d = nc.dram_tensor("vb", [FPC, 1], F32, kind="ExternalInput").ap()
    owT_d = nc.dram_tensor("owT", [FPC, D], MM_DT, kind="ExternalInput").ap()
    out_d = nc.dram_tensor("out", [B, S, D], MM_DT, kind="ExternalOutput").ap()

    # [B, D, S] with D split into 8 blocks of 128 partitions
    xT_r = xT_d.rearrange("b (o p) s -> b p o s", p=P)
    # fp8 x in DoubleRow pair layout: d = 256*o2 + 128*two + p
    xT8_r = xT8_d.rearrange("b (o2 two p) s -> b p o2 two s", two=2, p=P)
    # [B, S, D] with S split into 128-row blocks (partition-first)
    out_r = out_d.rearrange("b (o p) d -> b p o d", p=P)

    with tile.TileContext(nc) as tc:
        with (
            tc.tile_pool(name="singles", bufs=1) as singles,
            tc.tile_pool(name="xin", bufs=3) as xin,
            tc.tile_pool(name="x8in", bufs=3) as x8in,
            tc.tile_pool(name="ptile", bufs=5) as ptile,
            tc.tile_pool(name="small", bufs=4) as small,
            tc.tile_pool(name="outsb", bufs=3) as outsb,
        ):
            # --- constants / persistent tensors (loaded once) ---
            qw8_sb = singles.tile([P, DBLK2, 2, FPC], F8, tag="qw8")
            kw8_sb = singles.tile([P, DBLK2, 2, FPC], F8, tag="kw8")
            vwT_sb = singles.tile([P, DBLK, FPC], MM_DT, tag="vw")
            nc.sync.dma_start(
                out=qw8_sb,
                in_=qwT8_d.rearrange("(o2 two p) m -> p o2 two m", two=2, p=P))
            nc.sync.dma_start(
                out=kw8_sb,
                in_=kwT8_d.rearrange("(o2 two p) m -> p o2 two m", two=2, p=P))
            nc.sync.dma_start(out=vwT_sb, in_=vwT_d.rearrange("(o p) m -> p o m", p=P))
            qb_sb = singles.tile([FPC, 1], F32, tag="qb")
            kb_sb = singles.tile([FPC, 1], F32, tag="kb")
            vb_sb = singles.tile([FPC, 1], F32, tag="vb")
            nc.sync.dma_start(out=qb_sb, in_=qb_d)
            nc.sync.dma_start(out=kb_sb, in_=kb_d)
            nc.sync.dma_start(out=vb_sb, in_=vb_d)
            owT_sb = singles.tile([FPC, D], MM_DT, tag="ow")
            nc.sync.dma_start(out=owT_sb, in_=owT_d)

            qT_sb = singles.tile([P, B, S], MM_DT, tag="qT")
            kT_sb = singles.tile([P, B, S], MM_DT, tag="kT")
            vT_sb = singles.tile([P, B, S], MM_DT, tag="vT")
            oT_sb = singles.tile([P, B, S], MM_DT, tag="oT")
            # V_aug[t, b, h, tblk, 0:64] = v features (fp16, written
            # directly by the DMA XBAR transposes); [.., 64:128] = 1.0 ->
            # the PV matmul emits the softmax denominator replicated on
            # partitions 64:128
            v_aug = singles.tile([P, B, HPC, NTB, P], MM_DT, tag="vaug")
            ones_sb = singles.tile([P, 1], F32, tag="ones")
            nc.vector.memset(ones_sb, 1.0)
            nc.vector.tensor_copy(
                out=v_aug[:, :, :, :, HD:P],
                in_=ones_sb[:, 0][:, None, None, None, None].to_broadcast(
                    [P, B, HPC, NTB, HD]),
            )
            # static causal 0/1 mask (fp16) for the post-exp multiply:
            # m128[t, c] = (c >= t)
            m128 = singles.tile([P, P], MM_DT, tag="m128")
            nc.gpsimd.memset(m128, 1.0)
            nc.gpsimd.affine_select(
                out=m128, in_=m128, compare_op=mybir.AluOpType.is_ge,
                fill=0.0, base=0, pattern=[[1, P]], channel_multiplier=-1,
            )

            # PSUM pools span all repetitions so consecutive bodies
            # pipeline through slot rotation instead of draining at each
            # body boundary
            with (
                tc.tile_pool(name="pps", bufs=2, space="PSUM") as pps,
                tc.tile_pool(name="mpsum", bufs=2, space="PSUM") as mpsum,
                tc.tile_pool(name="opsum", bufs=2, space="PSUM") as opsum,
            ):
                # ------ repetitions (>1 only for HW timing calibration) --
                if hwloop and repeat > 1:
                    with tc.For_i(0, repeat) as _i:
                        for _u in range(unroll):
                            _emit_body(nc, tc, locals())
                else:
                    for _rep in range(repeat):
                        _emit_body(nc, tc, locals())

    return nc


def _emit_body(nc, tc, env):
    g = type("G", (), env)
    singles, xin, x8in, ptile, small, outsb = (
        g.singles, g.xin, g.x8in, g.ptile, g.small, g.outsb)
    qw8_sb, kw8_sb, vwT_sb = g.qw8_sb, g.kw8_sb, g.vwT_sb
    qb_sb, kb_sb, vb_sb, owT_sb = g.qb_sb, g.kb_sb, g.vb_sb, g.owT_sb
    qT_sb, kT_sb, vT_sb, oT_sb, v_aug = g.qT_sb, g.kT_sb, g.vT_sb, g.oT_sb, g.v_aug
    m128 = g.m128
    xT_r, xT8_r, out_r = g.xT_r, g.xT8_r, g.out_r
    pps, mpsum, opsum = g.pps, g.mpsum, g.opsum

    if True:
        def ph1_chunk(b, cn):
            # generator: yields between matmul groups so the emitter can
            # interleave projection work into the attention j-loop.
            xt8 = x8in.tile([P, DBLK2, 2, SQ_CHUNK], F8, tag="xt8",
                            name=f"xt8_{b}{cn}")
            for g2 in range(2):
                nc.sync.dma_start(
                    out=xt8[:, 2 * g2:2 * g2 + 2],
                    in_=xT8_r[b, :, 2 * g2:2 * g2 + 2, :, ts(cn, SQ_CHUNK)])
            xt = xin.tile([P, DBLK, SQ_CHUNK], MM_DT, tag="xt",
                          name=f"xt{b}{cn}")
            for qd in range(4):
                nc.sync.dma_start(
                    out=xt[:, 2 * qd:2 * qd + 2, :],
                    in_=xT_r[b, :, 2 * qd:2 * qd + 2, ts(cn, SQ_CHUNK)])
            # Q/K in fp8 DoubleRow (4 passes of 2x128 contraction)
            for w8_sb, bias_sb, dst in (
                (qw8_sb, qb_sb, qT_sb),
                (kw8_sb, kb_sb, kT_sb),
            ):
                ps = pps.tile([P, SQ_CHUNK], F32, tag="pps",
                              name=f"prj{b}{cn}{id(dst)%97}")
                for o2 in range(DBLK2):
                    nc.tensor.matmul(
                        ps,
                        lhsT=w8_sb[:, o2],
                        rhs=xt8[:, o2],
                        perf_mode=mybir.MatmulPerfMode.DoubleRow,
                        start=(o2 == 0),
                        stop=(o2 == DBLK2 - 1),
                    )
                    if o2 == 1:
                        yield
                # fused descale (1/4096) + bias add, fp32 PSUM -> fp16 SBUF
                nc.vector.tensor_scalar(
                    out=dst[:, b, ts(cn, SQ_CHUNK)], in0=ps,
                    scalar1=DESCALE, scalar2=bias_sb,
                    op0=mybir.AluOpType.mult, op1=mybir.AluOpType.add,
                )
                yield
            # V projection in fp16 (8 passes)
            ps = pps.tile([P, SQ_CHUNK], F32, tag="pps",
                          name=f"prjv{b}{cn}")
            for o in range(DBLK):
                nc.tensor.matmul(
                    ps,
                    lhsT=vwT_sb[:, o, :],
                    rhs=xt[:, o, :],
                    start=(o == 0),
                    stop=(o == DBLK - 1),
                )
                if o % 4 == 3:
                    yield
            nc.vector.tensor_scalar_add(
                out=vT_sb[:, b, ts(cn, SQ_CHUNK)], in0=ps, scalar1=vb_sb,
            )
            yield
            # move V of this chunk into PV-lhsT layout via DMA XBAR
            # transpose: [64 feat, 512 seq] -> [128 t x 4 blocks, 64 feat]
            for h in range(HPC):
                hs = h * HD
                nc.sync.dma_start(
                    out=v_aug[:, b, h, 4 * cn:4 * cn + 4, 0:HD],
                    in_=vT_sb[hs:hs + HD, b, ts(cn, SQ_CHUNK)],
                    transpose=True,
                )

        def att_chunk(b, i, po_h):
            # software-pipelined j-loop: QK+exp for j run one stage ahead
            # of PV(j-1), so the PE's in-order queue never parks on a PV
            # that waits for its exp — filler matmuls (projections, output
            # projections) slot in behind QK(j+1) instead.
            jmax = 4 * i + 3
            pend = None
            for j in range(jmax + 2):
                if j <= jmax:
                    # columns < 128k of diagonal blocks are fully masked;
                    # skip them in QK, exp and PV
                    k = j - 4 * i
                    col0 = P * k if k > 0 else 0
                    ps = mpsum.tile([P, HPC, SQ_CHUNK], F32, tag="ps",
                                    name=f"ps{b}{i}{j}")
                    # two heads' QK in adjacent PE row-tiles
                    for h in range(HPC):
                        hs = h * HD
                        nc.tensor.matmul(
                            ps[:, h, col0:],
                            lhsT=kT_sb[hs:hs + HD, b, ts(j, P)],
                            rhs=qT_sb[hs:hs + HD, b,
                                      i * SQ_CHUNK + col0:(i + 1) * SQ_CHUNK],
                            start=True,
                            stop=True,
                        )
                    pt = ptile.tile([P, HPC, SQ_CHUNK], MM_DT, tag="pt",
                                    name=f"pt{b}{i}{j}")
                    nc.scalar.activation(
                        out=pt[:, :, col0:], in_=ps[:, :, col0:],
                        func=mybir.ActivationFunctionType.Exp,
                        scale=0.125,
                    )
                    if k >= 0:
                        # causal zero-fill post-exp; lands a full stage
                        # before PV consumes the diagonal block
                        nc.vector.tensor_tensor(
                            out=pt[:, :, col0:col0 + P],
                            in0=pt[:, :, col0:col0 + P],
                            in1=m128[:, None, :].to_broadcast([P, HPC, P]),
                            op=mybir.AluOpType.mult,
                        )
                    cur = (j, col0, pt)
                else:
                    cur = None
                if pend is not None:
                    pj, pcol0, ppt = pend
                    for h in range(HPC):
                        nc.tensor.matmul(
                            po_h[h][:, pcol0:],
                            lhsT=v_aug[:, b, h, pj, :],
                            rhs=ppt[:, h, pcol0:],
                            start=(pj == 0),
                            stop=(pj == jmax),
                            skip_group_check=True,
                        )
                pend = cur
                yield

        def flush_norm(b, i, po_h):
            for h in range(HPC):
                hs = h * HD
                rb = small.tile([HD, SQ_CHUNK], F32, tag="rb",
                                name=f"rb{b}{i}{h}")
                nc.vector.reciprocal(out=rb, in_=po_h[h][HD:P, :])
                # split by 256-col halves so the first output-projection
                # s-blocks can start before the full chunk is normalized
                for q in range(2):
                    qs = q * (SQ_CHUNK // 2)
                    nc.vector.tensor_mul(
                        out=oT_sb[hs:hs + HD, b,
                                  i * SQ_CHUNK + qs:
                                  i * SQ_CHUNK + qs + SQ_CHUNK // 2],
                        in0=po_h[h][0:HD, qs:qs + SQ_CHUNK // 2],
                        in1=rb[:, qs:qs + SQ_CHUNK // 2],
                    )

        def flush_proj(b, i, norm_po=None, rbs=None, tail=False):
            # generator; when norm_po is given, the normalization of each
            # 256-col half is emitted just before the output-projection
            # matmuls that consume it, shrinking the serial chunk-boundary
            # section
            if rbs is None:
                rbs = [None, None]
            for half in range(2):
                if norm_po is not None:
                    qs = half * (SQ_CHUNK // 2)
                    for h in range(HPC):
                        hs = h * HD
                        if half == 0:
                            rb = small.tile([HD, SQ_CHUNK], F32, tag="rb",
                                            name=f"rb{b}{i}{h}")
                            rbs[h] = rb
                            nc.vector.reciprocal(
                                out=rb, in_=norm_po[h][HD:P, :])
                        nc.vector.tensor_mul(
                            out=oT_sb[hs:hs + HD, b,
                                      i * SQ_CHUNK + qs:
                                      i * SQ_CHUNK + qs + SQ_CHUNK // 2],
                            in0=norm_po[h][0:HD, qs:qs + SQ_CHUNK // 2],
                            in1=rbs[h][:, qs:qs + SQ_CHUNK // 2],
                        )
                ot = outsb.tile([P, 2, D], MM_DT, tag="ot",
                                name=f"ot{b}_{i}_{half}")
                for si in range(2):
                    s = 4 * i + 2 * half + si
                    for cc in range(2):
                        # post-attention (tail) flushes split both the PSUM
                        # pool (pps/mpsum) and the drain engine (DVE/ACT):
                        # the serial tail drain halves, and the next hwloop
                        # body's projections (gated on pps via DVE) and
                        # attention (gated on mpsum via ACT) both restart
                        # ~13us earlier. Inline flushes keep DVE-only (ACT
                        # is busy with the exp stream there).
                        odd = cc == 1
                        if tail and odd:
                            pw = mpsum.tile([P, HPC, SQ_CHUNK], F32,
                                            tag="ps", name=f"tp{b}_{s}_{cc}")
                            pp = pw[:, 0, :]
                        else:
                            pp = pps.tile([P, SQ_CHUNK], F32, tag="pps",
                                          name=f"pp{b}_{s}_{cc}")
                        nc.tensor.matmul(
                            pp,
                            lhsT=oT_sb[:, b, ts(s, P)],
                            rhs=owT_sb[:, ts(cc, SQ_CHUNK)],
                            start=True,
                            stop=True,
                        )
                        if odd:
                            nc.scalar.copy(
                                out=ot[:, si, ts(cc, SQ_CHUNK)], in_=pp,
                            )
                        else:
                            nc.vector.tensor_copy(
                                out=ot[:, si, ts(cc, SQ_CHUNK)], in_=pp,
                            )
                    yield
                # rows [s0, s0+1] of this batch as [128, 2, D]
                s0 = 4 * i + 2 * half
                nc.sync.dma_start(out=out_r[b, :, s0:s0 + 2, :], in_=ot)

        # chunk-interleaved schedule: the projection generator for the
        # next chunk is drained round-robin with the attention j-loop of
        # the previous chunk, so ready projection matmuls sit between
        # potentially-stalling QK ops in the PE queue
        chunks = [(b, cn) for b in range(B) for cn in range(NSQ)]
        groups = []
        for n, (b, cn) in enumerate(chunks):
            prev = chunks[n - 1] if n > 0 else None
            groups.append(((b, cn), prev))
        groups.append((None, chunks[-1]))

        deferred = []
        for gi, (pitem, aitem) in enumerate(groups):
            gp = ph1_chunk(*pitem) if pitem is not None else None
            if aitem is not None:
                b, i = aitem
                po_h = [
                    opsum.tile([P, SQ_CHUNK], F32, tag="po",
                               name=f"po{b}_{i}_{h}")
                    for h in range(HPC)
                ]
                ga = att_chunk(b, i, po_h)
            else:
                ga = None
            while gp is not None or ga is not None:
                if gp is not None:
                    try:
                        next(gp)
                    except StopIteration:
                        gp = None
                if ga is not None:
                    try:
                        next(ga)
                    except StopIteration:
                        ga = None
            if aitem is not None:
                # defer the last chunks' output projections so they can
                # fill the exp-paced tail of the final attention chunk
                if gi >= len(groups) - 5:
                    flush_norm(b, i, po_h)
                    deferred.append((b, i))
                else:
                    for _ in flush_proj(b, i, norm_po=po_h,
                                        rbs=[None, None]):
                        pass
        for b, i in deferred:
            for _ in flush_proj(b, i, tail=True):
                pass


def get_module(repeat=1, hwloop=False, unroll=1):
    key = ("nc", repeat, hwloop, unroll)
    if key not in _module_cache:
        m = _build_module(repeat=repeat, hwloop=hwloop, unroll=unroll)
        m.compile()
        _module_cache[key] = m
    return _module_cache[key]


def make_in_maps(x, qw, qb, kw, kb, vw, vb, ow):
    mmdt = np.dtype(np.float16)
    f8dt = np.dtype(mybir.dt.np(mybir.dt.float8e4))
    xT = np.ascontiguousarray(x.transpose(0, 2, 1)).astype(mmdt)  # [B, D, S]
    xT8 = np.ascontiguousarray(
        x.transpose(0, 2, 1).astype(np.float32) * X8_SCALE).astype(f8dt)
    in_maps = []
    for c in range(NCORES):
        sl = slice(c * FPC, (c + 1) * FPC)
        m = {
            "xT": xT,
            "xT8": xT8,
            "qwT8": np.ascontiguousarray(
                qw[sl, :].T.astype(np.float32) * W8_SCALE).astype(f8dt),
            "kwT8": np.ascontiguousarray(
                kw[sl, :].T.astype(np.float32) * W8_SCALE).astype(f8dt),
            "vwT": np.ascontiguousarray(vw[sl, :].T).astype(mmdt),
            "qb": np.ascontiguousarray(qb[sl].reshape(FPC, 1)).astype(np.float32),
            "kb": np.ascontiguousarray(kb[sl].reshape(FPC, 1)).astype(np.float32),
            "vb": np.ascontiguousarray(vb[sl].reshape(FPC, 1)).astype(np.float32),
            "owT": np.ascontiguousarray(ow[:, sl].T).astype(mmdt),
        }
        in_maps.append(m)
    return in_maps


def kernel(x, qw, qb, kw, kb, vw, vb, ow, ob, _trace=False):
    x = np.asarray(x, dtype=np.float32)
    qw = np.asarray(qw, dtype=np.float32)
    qb = np.asarray(qb, dtype=np.float32)
    kw = np.asarray(kw, dtype=np.float32)
    kb = np.asarray(kb, dtype=np.float32)
    vw = np.asarray(vw, dtype=np.float32)
    vb = np.asarray(vb, dtype=np.float32)
    ow = np.asarray(ow, dtype=np.float32)
    ob = np.asarray(ob, dtype=np.float32)

    nc = get_module()
    in_maps = make_in_maps(x, qw, qb, kw, kb, vw, vb, ow)
    res = run_bass_kernel_spmd(
        nc, in_maps, core_ids=list(range(NCORES)), trace=_trace
    )
    acc = np.zeros((B, S, D), dtype=np.float64)
    for r in res.results:
        acc += r["out"].astype(np.float64)
    out = (acc + ob.astype(np.float64)).astype(np.float32)
    if _trace:
        kernel.last_results = res
    return out


# revision 30
# speedup vs baseline: 1.0752x; 1.0156x over previous
"""Head-sharded (tensor-parallel) causal attention block for 8 NeuronCores.

Model: B=2, S=2048, D=1024, H=16 heads (HD=64). Each core owns 2 heads
(128 features) of the QKV projections and attention, computes a partial
output projection (o_shard @ ow_shard), and the host sums the 8 partials
and adds the output bias.

Layout (single PSUM scope, chunk-interleaved pipeline):
  - Q/K projections run in fp8e4 DoubleRow mode (2 k-tiles of 128 per
    pass -> 4 passes instead of 8, 2x PE throughput). Host supplies
    x*16 and qw*256/kw*256 in fp8; the PSUM result carries a 4096x
    scale that the fused bias-add (tensor_scalar mult+add) removes.
    V projection stays fp16 (v errors hit the output directly).
  - Projection and attention work interleave at j-tile granularity via
    generators: ready projection matmuls sit between potentially
    exp-stalled QK ops in the PE queue, so ScalarE's exp stream (the
    attention-phase pacer) overlaps projection matmuls.
  - V is projected into vT [feat, seq] (fp16), then moved to the PV
    lhsT layout V_aug[t, feat] via DMA XBAR transposes (no PE/PSUM).
  - V_aug columns 64:128 hold 1.0: the PV matmul emits the softmax
    denominator replicated on partitions 64:128, so normalization is a
    plain reciprocal + multiply (no partition broadcast).
  - The attention j-loop is software-pipelined: QK+exp for j are
    emitted one stage ahead of PV(j-1), so the in-order PE queue never
    parks on a PV waiting for its exp -- filler matmuls slot in behind
    the next QK instead. Causal mask: post-exp multiply of the diagonal
    128-block by a static 0/1 mask (DVE), emitted a full stage before
    PV consumes it. Fully-masked columns are skipped via col0 = 128*k.
  - Output projection partial[sq,1024] = oT.T @ owT in 512-wide halves
    through the projection PSUM pool; PSUM->SBUF copies ride DVE (ACT
    must stay free for the exp stream; GPSIMD cannot touch PSUM). The
    chunk's normalization (reciprocal of the PV-emitted denominator +
    multiply) is interleaved per 256-col half with the output
    projection that consumes it. The last chunks' projections are
    deferred past the final attention chunk.
  - PSUM budget (8 banks): proj/outproj 2x1, scores 2x2, PV accum 2x1.

Measured on the 8-core axon pod: ~199 us/body (baseline 219 us), rel
err 1.58e-2 vs the fp32 reference (gate 2e-2; the error is dominated
by the deliberate fp8 Q/K projections, measured identical in numpy
emulation).
"""

import numpy as np

import concourse.bass as bass
import concourse.mybir as mybir
import concourse.tile as tile
from concourse import bacc
from concourse.bass import ts
from concourse.bass_utils import run_bass_kernel_spmd

B, S, D, H = 2, 2048, 1024, 16
HD = D // H            # 64 head dim
NCORES = 8
FPC = D // NCORES      # 128 features per core
HPC = FPC // HD        # 2 heads per core
P = 128
SQ_CHUNK = 512         # query chunk (matmul free dim)
NSQ = S // SQ_CHUNK    # 4
NTB = S // P           # 16 t-blocks
DBLK = D // P          # 8 contraction blocks for fp16 projections
DBLK2 = DBLK // 2      # 4 DoubleRow passes for fp8 projections

F32 = mybir.dt.float32
MM_DT = mybir.dt.float16
F8 = mybir.dt.float8e4
X8_SCALE = 16.0        # x -> fp8 scale
W8_SCALE = 256.0       # qw/kw -> fp8 scale
DESCALE = 1.0 / (X8_SCALE * W8_SCALE)

_module_cache = {}


def _build_module(repeat=1, hwloop=False, unroll=1):
    nc = bacc.Bacc("TRN2", target_bir_lowering=False, debug=False)

    xT_d = nc.dram_tensor("xT", [B, D, S], MM_DT, kind="ExternalInput").ap()
    xT8_d = nc.dram_tensor("xT8", [B, D, S], F8, kind="ExternalInput").ap()
    qwT8_d = nc.dram_tensor("qwT8", [D, FPC], F8, kind="ExternalInput").ap()
    kwT8_d = nc.dram_tensor("kwT8", [D, FPC], F8, kind="ExternalInput").ap()
    vwT_d = nc.dram_tensor("vwT", [D, FPC], MM_DT, kind="ExternalInput").ap()
    qb_d = nc.dram_tensor("qb", [FPC, 1], F32, kind="ExternalInput").ap()
    kb_d = nc.dram_tensor("kb", [FPC, 1], F32, kind="ExternalInput").ap()
    vb_d = nc.dram_tensor("vb", [FPC, 1], F32, kind="ExternalInput").ap()
    owT_d = nc.dram_tensor("owT", [FPC, D], MM_DT, kind="ExternalInput").ap()
    out_d = nc.dram_tensor("out", [B, S, D], MM_DT, kind="ExternalOutput").ap()

    # [B, D, S] with D split into 8 blocks of 128 partitions
    xT_r = xT_d.rearrange("b (o p) s -> b p o s", p=P)
    # fp8 x in DoubleRow pair layout: d = 256*o2 + 128*two + p
    xT8_r = xT8_d.rearrange("b (o2 two p) s -> b p o2 two s", two=2, p=P)
    # [B, S, D] with S split into 128-row blocks (partition-first)
    out_r = out_d.rearrange("b (o p) d -> b p o d", p=P)

    with tile.TileContext(nc) as tc:
        with (
            tc.tile_pool(name="singles", bufs=1) as singles,
            tc.tile_pool(name="xin", bufs=3) as xin,
            tc.tile_pool(name="x8in", bufs=3) as x8in,
            tc.tile_pool(name="ptile", bufs=5) as ptile,
            tc.tile_pool(name="small", bufs=4) as small,
            tc.tile_pool(name="outsb", bufs=3) as outsb,
        ):
            # --- constants / persistent tensors (loaded once) ---
            qw8_sb = singles.tile([P, DBLK2, 2, FPC], F8, tag="qw8")
            kw8_sb = singles.tile([P, DBLK2, 2, FPC], F8, tag="kw8")
            vwT_sb = singles.tile([P, DBLK, FPC], MM_DT, tag="vw")
            nc.sync.dma_start(
                out=qw8_sb,
                in_=qwT8_d.rearrange("(o2 two p) m -> p o2 two m", two=2, p=P))
            nc.sync.dma_start(
                out=kw8_sb,
                in_=kwT8_d.rearrange("(o2 two p) m -> p o2 two m", two=2, p=P))
            nc.sync.dma_start(out=vwT_sb, in_=vwT_d.rearrange("(o p) m -> p o m", p=P))
            qb_sb = singles.tile([FPC, 1], F32, tag="qb")
            kb_sb = singles.tile([FPC, 1], F32, tag="kb")
            vb_sb = singles.tile([FPC, 1], F32, tag="vb")
            nc.sync.dma_start(out=qb_sb, in_=qb_d)
            nc.sync.dma_start(out=kb_sb, in_=kb_d)
            nc.sync.dma_start(out=vb_sb, in_=vb_d)
            owT_sb = singles.tile([FPC, D], MM_DT, tag="ow")
            nc.sync.dma_start(out=owT_sb, in_=owT_d)

            qT_sb = singles.tile([P, B, S], MM_DT, tag="qT")
            kT_sb = singles.tile([P, B, S], MM_DT, tag="kT")
            vT_sb = singles.tile([P, B, S], MM_DT, tag="vT")
            oT_sb = singles.tile([P, B, S], MM_DT, tag="oT")
            # V_aug[t, b, h, tblk, 0:64] = v features (fp16, written
            # directly by the DMA XBAR transposes); [.., 64:128] = 1.0 ->
            # the PV matmul emits the softmax denominator replicated on
            # partitions 64:128
            v_aug = singles.tile([P, B, HPC, NTB, P], MM_DT, tag="vaug")
            ones_sb = singles.tile([P, 1], F32, tag="ones")
            nc.vector.memset(ones_sb, 1.0)
            nc.vector.tensor_copy(
                out=v_aug[:, :, :, :, HD:P],
                in_=ones_sb[:, 0][:, None, None, None, None].to_broadcast(
                    [P, B, HPC, NTB, HD]),
            )
            # static causal 0/1 mask (fp16) for the post-exp multiply:
            # m128[t, c] = (c >= t)
            m128 = singles.tile([P, P], MM_DT, tag="m128")
            nc.gpsimd.memset(m128, 1.0)
            nc.gpsimd.affine_select(
                out=m128, in_=m128, compare_op=mybir.AluOpType.is_ge,
                fill=0.0, base=0, pattern=[[1, P]], channel_multiplier=-1,
            )

            # PSUM pools span all repetitions so consecutive bodies
            # pipeline through slot rotation instead of draining at each
            # body boundary
            with (
                tc.tile_pool(name="pps", bufs=2, space="PSUM") as pps,
                tc.tile_pool(name="mpsum", bufs=2, space="PSUM") as mpsum,
                tc.tile_pool(name="opsum", bufs=2, space="PSUM") as opsum,
            ):
                # ------ repetitions (>1 only for HW timing calibration) --
                if hwloop and repeat > 1:
                    with tc.For_i(0, repeat) as _i:
                        for _u in range(unroll):
                            _emit_body(nc, tc, locals())
                else:
                    for _rep in range(repeat):
                        _emit_body(nc, tc, locals())

    return nc


def _emit_body(nc, tc, env):
    g = type("G", (), env)
    singles, xin, x8in, ptile, small, outsb = (
        g.singles, g.xin, g.x8in, g.ptile, g.small, g.outsb)
    qw8_sb, kw8_sb, vwT_sb = g.qw8_sb, g.kw8_sb, g.vwT_sb
    qb_sb, kb_sb, vb_sb, owT_sb = g.qb_sb, g.kb_sb, g.vb_sb, g.owT_sb
    qT_sb, kT_sb, vT_sb, oT_sb, v_aug = g.qT_sb, g.kT_sb, g.vT_sb, g.oT_sb, g.v_aug
    m128 = g.m128
    xT_r, xT8_r, out_r = g.xT_r, g.xT8_r, g.out_r
    pps, mpsum, opsum = g.pps, g.mpsum, g.opsum

    if True:
        def ph1_chunk(b, cn):
            # generator: yields between matmul groups so the emitter can
            # interleave projection work into the attention j-loop.
            xt8 = x8in.tile([P, DBLK2, 2, SQ_CHUNK], F8, tag="xt8",
                            name=f"xt8_{b}{cn}")
            for g2 in range(2):
                nc.sync.dma_start(
                    out=xt8[:, 2 * g2:2 * g2 + 2],
                    in_=xT8_r[b, :, 2 * g2:2 * g2 + 2, :, ts(cn, SQ_CHUNK)])
            xt = xin.tile([P, DBLK, SQ_CHUNK], MM_DT, tag="xt",
                          name=f"xt{b}{cn}")
            for qd in range(4):
                nc.sync.dma_start(
                    out=xt[:, 2 * qd:2 * qd + 2, :],
                    in_=xT_r[b, :, 2 * qd:2 * qd + 2, ts(cn, SQ_CHUNK)])
            # Q/K in fp8 DoubleRow (4 passes of 2x128 contraction)
            for w8_sb, bias_sb, dst in (
                (qw8_sb, qb_sb, qT_sb),
                (kw8_sb, kb_sb, kT_sb),
            ):
                ps = pps.tile([P, SQ_CHUNK], F32, tag="pps",
                              name=f"prj{b}{cn}{id(dst)%97}")
                for o2 in range(DBLK2):
                    nc.tensor.matmul(
                        ps,
                        lhsT=w8_sb[:, o2],
                        rhs=xt8[:, o2],
                        perf_mode=mybir.MatmulPerfMode.DoubleRow,
                        start=(o2 == 0),
                        stop=(o2 == DBLK2 - 1),
                    )
                    if o2 == 1:
                        yield
                # fused descale (1/4096) + bias add, fp32 PSUM -> fp16 SBUF
                nc.vector.tensor_scalar(
                    out=dst[:, b, ts(cn, SQ_CHUNK)], in0=ps,
                    scalar1=DESCALE, scalar2=bias_sb,
                    op0=mybir.AluOpType.mult, op1=mybir.AluOpType.add,
                )
                yield
            # V projection in fp16 (8 passes)
            ps = pps.tile([P, SQ_CHUNK], F32, tag="pps",
                          name=f"prjv{b}{cn}")
            for o in range(DBLK):
                nc.tensor.matmul(
                    ps,
                    lhsT=vwT_sb[:, o, :],
                    rhs=xt[:, o, :],
                    start=(o == 0),
                    stop=(o == DBLK - 1),
                )
                if o % 4 == 3:
                    yield
            nc.vector.tensor_scalar_add(
                out=vT_sb[:, b, ts(cn, SQ_CHUNK)], in0=ps, scalar1=vb_sb,
            )
            yield
            # move V of this chunk into PV-lhsT layout via DMA XBAR
            # transpose: [64 feat, 512 seq] -> [128 t x 4 blocks, 64 feat]
            for h in range(HPC):
                hs = h * HD
                nc.sync.dma_start(
                    out=v_aug[:, b, h, 4 * cn:4 * cn + 4, 0:HD],
                    in_=vT_sb[hs:hs + HD, b, ts(cn, SQ_CHUNK)],
                    transpose=True,
                )

        def att_chunk(b, i, po_h):
            # software-pipelined j-loop: QK+exp for j run one stage ahead
            # of PV(j-1), so the PE's in-order queue never parks on a PV
            # that waits for its exp — filler matmuls (projections, output
            # projections) slot in behind QK(j+1) instead.
            jmax = 4 * i + 3
            pend = None
            for j in range(jmax + 2):
                if j <= jmax:
                    # columns < 128k of diagonal blocks are fully masked;
                    # skip them in QK, exp and PV
                    k = j - 4 * i
                    col0 = P * k if k > 0 else 0
                    ps = mpsum.tile([P, HPC, SQ_CHUNK], F32, tag="ps",
                                    name=f"ps{b}{i}{j}")
                    # two heads' QK in adjacent PE row-tiles
                    for h in range(HPC):
                        hs = h * HD
                        nc.tensor.matmul(
                            ps[:, h, col0:],
                            lhsT=kT_sb[hs:hs + HD, b, ts(j, P)],
                            rhs=qT_sb[hs:hs + HD, b,
                                      i * SQ_CHUNK + col0:(i + 1) * SQ_CHUNK],
                            start=True,
                            stop=True,
                        )
                    pt = ptile.tile([P, HPC, SQ_CHUNK], MM_DT, tag="pt",
                                    name=f"pt{b}{i}{j}")
                    nc.scalar.activation(
                        out=pt[:, :, col0:], in_=ps[:, :, col0:],
                        func=mybir.ActivationFunctionType.Exp,
                        scale=0.125,
                    )
                    if k >= 0:
                        # causal zero-fill post-exp; lands a full stage
                        # before PV consumes the diagonal block
                        nc.vector.tensor_tensor(
                            out=pt[:, :, col0:col0 + P],
                            in0=pt[:, :, col0:col0 + P],
                            in1=m128[:, None, :].to_broadcast([P, HPC, P]),
                            op=mybir.AluOpType.mult,
                        )
                    cur = (j, col0, pt)
                else:
                    cur = None
                if pend is not None:
                    pj, pcol0, ppt = pend
                    for h in range(HPC):
                        nc.tensor.matmul(
                            po_h[h][:, pcol0:],
                            lhsT=v_aug[:, b, h, pj, :],
                            rhs=ppt[:, h, pcol0:],
                            start=(pj == 0),
                            stop=(pj == jmax),
                            skip_group_check=True,
                        )
                pend = cur
                yield

        def flush_norm(b, i, po_h):
            for h in range(HPC):
                hs = h * HD
                rb = small.tile([HD, SQ_CHUNK], F32, tag="rb",
                                name=f"rb{b}{i}{h}")
                nc.vector.reciprocal(out=rb, in_=po_h[h][HD:P, :])
                # split by 256-col halves so the first output-projection
                # s-blocks can start before the full chunk is normalized
                for q in range(2):
                    qs = q * (SQ_CHUNK // 2)
                    nc.vector.tensor_mul(
                        out=oT_sb[hs:hs + HD, b,
                                  i * SQ_CHUNK + qs:
                                  i * SQ_CHUNK + qs + SQ_CHUNK // 2],
                        in0=po_h[h][0:HD, qs:qs + SQ_CHUNK // 2],
                        in1=rb[:, qs:qs + SQ_CHUNK // 2],
                    )

        def flush_proj(b, i, norm_po=None, rbs=None, tail=False):
            # generator; when norm_po is given, the normalization of each
            # 256-col half is emitted just before the output-projection
            # matmuls that consume it, shrinking the serial chunk-boundary
            # section
            if rbs is None:
                rbs = [None, None]
            for half in range(2):
                if norm_po is not None:
                    qs = half * (SQ_CHUNK // 2)
                    for h in range(HPC):
                        hs = h * HD
                        if half == 0:
                            rb = small.tile([HD, SQ_CHUNK], F32, tag="rb",
                                            name=f"rb{b}{i}{h}")
                            rbs[h] = rb
                            nc.vector.reciprocal(
                                out=rb, in_=norm_po[h][HD:P, :])
                        nc.vector.tensor_mul(
                            out=oT_sb[hs:hs + HD, b,
                                      i * SQ_CHUNK + qs:
                                      i * SQ_CHUNK + qs + SQ_CHUNK // 2],
                            in0=norm_po[h][0:HD, qs:qs + SQ_CHUNK // 2],
                            in1=rbs[h][:, qs:qs + SQ_CHUNK // 2],
                        )
                ot = outsb.tile([P, 2, D], MM_DT, tag="ot",
                                name=f"ot{b}_{i}_{half}")
                for si in range(2):
                    s = 4 * i + 2 * half + si
                    for cc in range(2):
                        # post-attention (tail) flushes split both the PSUM
                        # pool (pps/mpsum) and the drain engine (DVE/ACT):
                        # the serial tail drain halves, and the next hwloop
                        # body's projections (gated on pps via DVE) and
                        # attention (gated on mpsum via ACT) both restart
                        # ~13us earlier. Inline flushes keep DVE-only (ACT
                        # is busy with the exp stream there).
                        odd = cc == 1
                        if tail and odd:
                            pw = mpsum.tile([P, HPC, SQ_CHUNK], F32,
                                            tag="ps", name=f"tp{b}_{s}_{cc}")
                            pp = pw[:, 0, :]
                        else:
                            pp = pps.tile([P, SQ_CHUNK], F32, tag="pps",
                                          name=f"pp{b}_{s}_{cc}")
                        nc.tensor.matmul(
                            pp,
                            lhsT=oT_sb[:, b, ts(s, P)],
                            rhs=owT_sb[:, ts(cc, SQ_CHUNK)],
                            start=True,
                            stop=True,
                        )
                        if odd:
                            nc.scalar.copy(
                                out=ot[:, si, ts(cc, SQ_CHUNK)], in_=pp,
                            )
                        else:
                            nc.vector.tensor_copy(
                                out=ot[:, si, ts(cc, SQ_CHUNK)], in_=pp,
                            )
                    yield
                # rows [s0, s0+1] of this batch as [128, 2, D]
                s0 = 4 * i + 2 * half
                nc.sync.dma_start(out=out_r[b, :, s0:s0 + 2, :], in_=ot)

        # chunk-interleaved schedule: the projection generator for the
        # next chunk is drained round-robin with the attention j-loop of
        # the previous chunk, so ready projection matmuls sit between
        # potentially-stalling QK ops in the PE queue
        chunks = [(b, cn) for b in range(B) for cn in range(NSQ)]
        groups = []
        for n, (b, cn) in enumerate(chunks):
            prev = chunks[n - 1] if n > 0 else None
            groups.append(((b, cn), prev))
        groups.append((None, chunks[-1]))

        deferred = []
        for gi, (pitem, aitem) in enumerate(groups):
            gp = ph1_chunk(*pitem) if pitem is not None else None
            if aitem is not None:
                b, i = aitem
                po_h = [
                    opsum.tile([P, SQ_CHUNK], F32, tag="po",
                               name=f"po{b}_{i}_{h}")
                    for h in range(HPC)
                ]
                ga = att_chunk(b, i, po_h)
            else:
                ga = None
            while gp is not None or ga is not None:
                if gp is not None:
                    try:
                        next(gp)
                    except StopIteration:
                        gp = None
                if ga is not None:
                    try:
                        next(ga)
                    except StopIteration:
                        ga = None
            if aitem is not None:
                # defer the last chunks' output projections so they can
                # fill the exp-paced tail of the final attention chunk
                if gi >= len(groups) - 7:
                    flush_norm(b, i, po_h)
                    deferred.append((b, i))
                else:
                    for _ in flush_proj(b, i, norm_po=po_h,
                                        rbs=[None, None]):
                        pass
        for b, i in deferred:
            for _ in flush_proj(b, i, tail=True):
                pass


def get_module(repeat=1, hwloop=False, unroll=1):
    key = ("nc", repeat, hwloop, unroll)
    if key not in _module_cache:
        m = _build_module(repeat=repeat, hwloop=hwloop, unroll=unroll)
        m.compile()
        _module_cache[key] = m
    return _module_cache[key]


def make_in_maps(x, qw, qb, kw, kb, vw, vb, ow):
    mmdt = np.dtype(np.float16)
    f8dt = np.dtype(mybir.dt.np(mybir.dt.float8e4))
    xT = np.ascontiguousarray(x.transpose(0, 2, 1)).astype(mmdt)  # [B, D, S]
    xT8 = np.ascontiguousarray(
        x.transpose(0, 2, 1).astype(np.float32) * X8_SCALE).astype(f8dt)
    in_maps = []
    for c in range(NCORES):
        sl = slice(c * FPC, (c + 1) * FPC)
        m = {
            "xT": xT,
            "xT8": xT8,
            "qwT8": np.ascontiguousarray(
                qw[sl, :].T.astype(np.float32) * W8_SCALE).astype(f8dt),
            "kwT8": np.ascontiguousarray(
                kw[sl, :].T.astype(np.float32) * W8_SCALE).astype(f8dt),
            "vwT": np.ascontiguousarray(vw[sl, :].T).astype(mmdt),
            "qb": np.ascontiguousarray(qb[sl].reshape(FPC, 1)).astype(np.float32),
            "kb": np.ascontiguousarray(kb[sl].reshape(FPC, 1)).astype(np.float32),
            "vb": np.ascontiguousarray(vb[sl].reshape(FPC, 1)).astype(np.float32),
            "owT": np.ascontiguousarray(ow[:, sl].T).astype(mmdt),
        }
        in_maps.append(m)
    return in_maps


def kernel(x, qw, qb, kw, kb, vw, vb, ow, ob, _trace=False):
    x = np.asarray(x, dtype=np.float32)
    qw = np.asarray(qw, dtype=np.float32)
    qb = np.asarray(qb, dtype=np.float32)
    kw = np.asarray(kw, dtype=np.float32)
    kb = np.asarray(kb, dtype=np.float32)
    vw = np.asarray(vw, dtype=np.float32)
    vb = np.asarray(vb, dtype=np.float32)
    ow = np.asarray(ow, dtype=np.float32)
    ob = np.asarray(ob, dtype=np.float32)

    nc = get_module()
    in_maps = make_in_maps(x, qw, qb, kw, kb, vw, vb, ow)
    res = run_bass_kernel_spmd(
        nc, in_maps, core_ids=list(range(NCORES)), trace=_trace
    )
    acc = np.zeros((B, S, D), dtype=np.float64)
    for r in res.results:
        acc += r["out"].astype(np.float64)
    out = (acc + ob.astype(np.float64)).astype(np.float32)
    if _trace:
        kernel.last_results = res
    return out


# revision 32
# speedup vs baseline: 1.0767x; 1.0014x over previous
"""Head-sharded (tensor-parallel) causal attention block for 8 NeuronCores.

Model: B=2, S=2048, D=1024, H=16 heads (HD=64). Each core owns 2 heads
(128 features) of the QKV projections and attention, computes a partial
output projection (o_shard @ ow_shard), and the host sums the 8 partials
and adds the output bias.

Layout (single PSUM scope, chunk-interleaved pipeline):
  - Q/K projections run in fp8e4 DoubleRow mode (2 k-tiles of 128 per
    pass -> 4 passes instead of 8, 2x PE throughput). Host supplies
    x*16 and qw*256/kw*256 in fp8; the PSUM result carries a 4096x
    scale that the fused bias-add (tensor_scalar mult+add) removes.
    V projection stays fp16 (v errors hit the output directly).
  - Projection and attention work interleave at j-tile granularity via
    generators: ready projection matmuls sit between potentially
    exp-stalled QK ops in the PE queue, so ScalarE's exp stream (the
    attention-phase pacer) overlaps projection matmuls.
  - V is projected into vT [feat, seq] (fp16), then moved to the PV
    lhsT layout V_aug[t, feat] via DMA XBAR transposes (no PE/PSUM).
  - V_aug columns 64:128 hold 1.0: the PV matmul emits the softmax
    denominator replicated on partitions 64:128, so normalization is a
    plain reciprocal + multiply (no partition broadcast).
  - The attention j-loop is software-pipelined: QK+exp for j are
    emitted one stage ahead of PV(j-1), so the in-order PE queue never
    parks on a PV waiting for its exp -- filler matmuls slot in behind
    the next QK instead. Causal mask: post-exp multiply of the diagonal
    128-block by a static 0/1 mask (DVE), emitted a full stage before
    PV consumes it. Fully-masked columns are skipped via col0 = 128*k.
  - Output projection partial[sq,1024] = oT.T @ owT in 512-wide halves
    through the projection PSUM pool; PSUM->SBUF copies ride DVE (ACT
    must stay free for the exp stream; GPSIMD cannot touch PSUM). The
    chunk's normalization (reciprocal of the PV-emitted denominator +
    multiply) is interleaved per 256-col half with the output
    projection that consumes it. The last chunks' projections are
    deferred past the final attention chunk.
  - PSUM budget (8 banks): proj/outproj 2x1, scores 2x2, PV accum 2x1.

Measured on the 8-core axon pod: ~199 us/body (baseline 219 us), rel
err 1.58e-2 vs the fp32 reference (gate 2e-2; the error is dominated
by the deliberate fp8 Q/K projections, measured identical in numpy
emulation).
"""

import numpy as np

import concourse.bass as bass
import concourse.mybir as mybir
import concourse.tile as tile
from concourse import bacc
from concourse.bass import ts
from concourse.bass_utils import run_bass_kernel_spmd

B, S, D, H = 2, 2048, 1024, 16
HD = D // H            # 64 head dim
NCORES = 8
FPC = D // NCORES      # 128 features per core
HPC = FPC // HD        # 2 heads per core
P = 128
SQ_CHUNK = 512         # query chunk (matmul free dim)
NSQ = S // SQ_CHUNK    # 4
NTB = S // P           # 16 t-blocks
DBLK = D // P          # 8 contraction blocks for fp16 projections
DBLK2 = DBLK // 2      # 4 DoubleRow passes for fp8 projections

F32 = mybir.dt.float32
MM_DT = mybir.dt.float16
F8 = mybir.dt.float8e4
X8_SCALE = 16.0        # x -> fp8 scale
W8_SCALE = 256.0       # qw/kw -> fp8 scale
DESCALE = 1.0 / (X8_SCALE * W8_SCALE)

_module_cache = {}


def _build_module(repeat=1, hwloop=False, unroll=1):
    nc = bacc.Bacc("TRN2", target_bir_lowering=False, debug=False)

    xT_d = nc.dram_tensor("xT", [B, D, S], MM_DT, kind="ExternalInput").ap()
    xT8_d = nc.dram_tensor("xT8", [B, D, S], F8, kind="ExternalInput").ap()
    qwT8_d = nc.dram_tensor("qwT8", [D, FPC], F8, kind="ExternalInput").ap()
    kwT8_d = nc.dram_tensor("kwT8", [D, FPC], F8, kind="ExternalInput").ap()
    vwT_d = nc.dram_tensor("vwT", [D, FPC], MM_DT, kind="ExternalInput").ap()
    qb_d = nc.dram_tensor("qb", [FPC, 1], F32, kind="ExternalInput").ap()
    kb_d = nc.dram_tensor("kb", [FPC, 1], F32, kind="ExternalInput").ap()
    vb_d = nc.dram_tensor("vb", [FPC, 1], F32, kind="ExternalInput").ap()
    owT_d = nc.dram_tensor("owT", [FPC, D], MM_DT, kind="ExternalInput").ap()
    out_d = nc.dram_tensor("out", [B, S, D], MM_DT, kind="ExternalOutput").ap()

    # [B, D, S] with D split into 8 blocks of 128 partitions
    xT_r = xT_d.rearrange("b (o p) s -> b p o s", p=P)
    # fp8 x in DoubleRow pair layout: d = 256*o2 + 128*two + p
    xT8_r = xT8_d.rearrange("b (o2 two p) s -> b p o2 two s", two=2, p=P)
    # [B, S, D] with S split into 128-row blocks (partition-first)
    out_r = out_d.rearrange("b (o p) d -> b p o d", p=P)

    with tile.TileContext(nc) as tc:
        with (
            tc.tile_pool(name="singles", bufs=1) as singles,
            tc.tile_pool(name="xin", bufs=3) as xin,
            tc.tile_pool(name="x8in", bufs=3) as x8in,
            tc.tile_pool(name="ptile", bufs=5) as ptile,
            tc.tile_pool(name="small", bufs=4) as small,
            tc.tile_pool(name="outsb", bufs=3) as outsb,
        ):
            # --- constants / persistent tensors (loaded once) ---
            qw8_sb = singles.tile([P, DBLK2, 2, FPC], F8, tag="qw8")
            kw8_sb = singles.tile([P, DBLK2, 2, FPC], F8, tag="kw8")
            vwT_sb = singles.tile([P, DBLK, FPC], MM_DT, tag="vw")
            nc.sync.dma_start(
                out=qw8_sb,
                in_=qwT8_d.rearrange("(o2 two p) m -> p o2 two m", two=2, p=P))
            nc.sync.dma_start(
                out=kw8_sb,
                in_=kwT8_d.rearrange("(o2 two p) m -> p o2 two m", two=2, p=P))
            nc.sync.dma_start(out=vwT_sb, in_=vwT_d.rearrange("(o p) m -> p o m", p=P))
            qb_sb = singles.tile([FPC, 1], F32, tag="qb")
            kb_sb = singles.tile([FPC, 1], F32, tag="kb")
            vb_sb = singles.tile([FPC, 1], F32, tag="vb")
            nc.sync.dma_start(out=qb_sb, in_=qb_d)
            nc.sync.dma_start(out=kb_sb, in_=kb_d)
            nc.sync.dma_start(out=vb_sb, in_=vb_d)
            owT_sb = singles.tile([FPC, D], MM_DT, tag="ow")
            nc.sync.dma_start(out=owT_sb, in_=owT_d)

            qT_sb = singles.tile([P, B, S], MM_DT, tag="qT")
            kT_sb = singles.tile([P, B, S], MM_DT, tag="kT")
            vT_sb = singles.tile([P, B, S], MM_DT, tag="vT")
            oT_sb = singles.tile([P, B, S], MM_DT, tag="oT")
            # V_aug[t, b, h, tblk, 0:64] = v features (fp16, written
            # directly by the DMA XBAR transposes); [.., 64:128] = 1.0 ->
            # the PV matmul emits the softmax denominator replicated on
            # partitions 64:128
            v_aug = singles.tile([P, B, HPC, NTB, P], MM_DT, tag="vaug")
            ones_sb = singles.tile([P, 1], F32, tag="ones")
            nc.vector.memset(ones_sb, 1.0)
            nc.vector.tensor_copy(
                out=v_aug[:, :, :, :, HD:P],
                in_=ones_sb[:, 0][:, None, None, None, None].to_broadcast(
                    [P, B, HPC, NTB, HD]),
            )
            # static causal 0/1 mask (fp16) for the post-exp multiply:
            # m128[t, c] = (c >= t)
            m128 = singles.tile([P, P], MM_DT, tag="m128")
            nc.gpsimd.memset(m128, 1.0)
            nc.gpsimd.affine_select(
                out=m128, in_=m128, compare_op=mybir.AluOpType.is_ge,
                fill=0.0, base=0, pattern=[[1, P]], channel_multiplier=-1,
            )

            # PSUM pools span all repetitions so consecutive bodies
            # pipeline through slot rotation instead of draining at each
            # body boundary
            with (
                tc.tile_pool(name="pps", bufs=2, space="PSUM") as pps,
                tc.tile_pool(name="mpsum", bufs=2, space="PSUM") as mpsum,
                tc.tile_pool(name="opsum", bufs=2, space="PSUM") as opsum,
            ):
                # ------ repetitions (>1 only for HW timing calibration) --
                if hwloop and repeat > 1:
                    with tc.For_i(0, repeat) as _i:
                        for _u in range(unroll):
                            _emit_body(nc, tc, locals())
                else:
                    for _rep in range(repeat):
                        _emit_body(nc, tc, locals())

    return nc


def _emit_body(nc, tc, env):
    g = type("G", (), env)
    singles, xin, x8in, ptile, small, outsb = (
        g.singles, g.xin, g.x8in, g.ptile, g.small, g.outsb)
    qw8_sb, kw8_sb, vwT_sb = g.qw8_sb, g.kw8_sb, g.vwT_sb
    qb_sb, kb_sb, vb_sb, owT_sb = g.qb_sb, g.kb_sb, g.vb_sb, g.owT_sb
    qT_sb, kT_sb, vT_sb, oT_sb, v_aug = g.qT_sb, g.kT_sb, g.vT_sb, g.oT_sb, g.v_aug
    m128 = g.m128
    xT_r, xT8_r, out_r = g.xT_r, g.xT8_r, g.out_r
    pps, mpsum, opsum = g.pps, g.mpsum, g.opsum

    if True:
        def ph1_chunk(b, cn):
            # generator: yields between matmul groups so the emitter can
            # interleave projection work into the attention j-loop.
            xt8 = x8in.tile([P, DBLK2, 2, SQ_CHUNK], F8, tag="xt8",
                            name=f"xt8_{b}{cn}")
            for g2 in range(2):
                nc.sync.dma_start(
                    out=xt8[:, 2 * g2:2 * g2 + 2],
                    in_=xT8_r[b, :, 2 * g2:2 * g2 + 2, :, ts(cn, SQ_CHUNK)])
            xt = xin.tile([P, DBLK, SQ_CHUNK], MM_DT, tag="xt",
                          name=f"xt{b}{cn}")
            for qd in range(4):
                nc.sync.dma_start(
                    out=xt[:, 2 * qd:2 * qd + 2, :],
                    in_=xT_r[b, :, 2 * qd:2 * qd + 2, ts(cn, SQ_CHUNK)])
            # Q/K in fp8 DoubleRow (4 passes of 2x128 contraction)
            for w8_sb, bias_sb, dst in (
                (qw8_sb, qb_sb, qT_sb),
                (kw8_sb, kb_sb, kT_sb),
            ):
                ps = pps.tile([P, SQ_CHUNK], F32, tag="pps",
                              name=f"prj{b}{cn}{id(dst)%97}")
                for o2 in range(DBLK2):
                    nc.tensor.matmul(
                        ps,
                        lhsT=w8_sb[:, o2],
                        rhs=xt8[:, o2],
                        perf_mode=mybir.MatmulPerfMode.DoubleRow,
                        start=(o2 == 0),
                        stop=(o2 == DBLK2 - 1),
                    )
                    if o2 == 1:
                        yield
                # fused descale (1/4096) + bias add, fp32 PSUM -> fp16
                # SBUF, on ACT (out = Copy(in*scale + bias)) to unload DVE
                nc.scalar.activation(
                    out=dst[:, b, ts(cn, SQ_CHUNK)], in_=ps,
                    func=mybir.ActivationFunctionType.Identity,
                    scale=DESCALE, bias=bias_sb,
                )
                yield
            # V projection in fp16 (8 passes)
            ps = pps.tile([P, SQ_CHUNK], F32, tag="pps",
                          name=f"prjv{b}{cn}")
            for o in range(DBLK):
                nc.tensor.matmul(
                    ps,
                    lhsT=vwT_sb[:, o, :],
                    rhs=xt[:, o, :],
                    start=(o == 0),
                    stop=(o == DBLK - 1),
                )
                if o % 4 == 3:
                    yield
            nc.vector.tensor_scalar_add(
                out=vT_sb[:, b, ts(cn, SQ_CHUNK)], in0=ps, scalar1=vb_sb,
            )
            yield
            # move V of this chunk into PV-lhsT layout via DMA XBAR
            # transpose: [64 feat, 512 seq] -> [128 t x 4 blocks, 64 feat]
            for h in range(HPC):
                hs = h * HD
                nc.sync.dma_start(
                    out=v_aug[:, b, h, 4 * cn:4 * cn + 4, 0:HD],
                    in_=vT_sb[hs:hs + HD, b, ts(cn, SQ_CHUNK)],
                    transpose=True,
                )

        def att_chunk(b, i, po_h):
            # software-pipelined j-loop: QK+exp for j run one stage ahead
            # of PV(j-1), so the PE's in-order queue never parks on a PV
            # that waits for its exp — filler matmuls (projections, output
            # projections) slot in behind QK(j+1) instead.
            jmax = 4 * i + 3
            pend = None
            for j in range(jmax + 2):
                if j <= jmax:
                    # columns < 128k of diagonal blocks are fully masked;
                    # skip them in QK, exp and PV
                    k = j - 4 * i
                    col0 = P * k if k > 0 else 0
                    ps = mpsum.tile([P, HPC, SQ_CHUNK], F32, tag="ps",
                                    name=f"ps{b}{i}{j}")
                    # two heads' QK in adjacent PE row-tiles
                    for h in range(HPC):
                        hs = h * HD
                        nc.tensor.matmul(
                            ps[:, h, col0:],
                            lhsT=kT_sb[hs:hs + HD, b, ts(j, P)],
                            rhs=qT_sb[hs:hs + HD, b,
                                      i * SQ_CHUNK + col0:(i + 1) * SQ_CHUNK],
                            start=True,
                            stop=True,
                        )
                    pt = ptile.tile([P, HPC, SQ_CHUNK], MM_DT, tag="pt",
                                    name=f"pt{b}{i}{j}")
                    nc.scalar.activation(
                        out=pt[:, :, col0:], in_=ps[:, :, col0:],
                        func=mybir.ActivationFunctionType.Exp,
                        scale=0.125,
                    )
                    if k >= 0:
                        # causal zero-fill post-exp; lands a full stage
                        # before PV consumes the diagonal block
                        nc.vector.tensor_tensor(
                            out=pt[:, :, col0:col0 + P],
                            in0=pt[:, :, col0:col0 + P],
                            in1=m128[:, None, :].to_broadcast([P, HPC, P]),
                            op=mybir.AluOpType.mult,
                        )
                    cur = (j, col0, pt)
                else:
                    cur = None
                if pend is not None:
                    pj, pcol0, ppt = pend
                    for h in range(HPC):
                        nc.tensor.matmul(
                            po_h[h][:, pcol0:],
                            lhsT=v_aug[:, b, h, pj, :],
                            rhs=ppt[:, h, pcol0:],
                            start=(pj == 0),
                            stop=(pj == jmax),
                            skip_group_check=True,
                        )
                pend = cur
                yield

        def flush_norm(b, i, po_h):
            for h in range(HPC):
                hs = h * HD
                rb = small.tile([HD, SQ_CHUNK], F32, tag="rb",
                                name=f"rb{b}{i}{h}")
                nc.vector.reciprocal(out=rb, in_=po_h[h][HD:P, :])
                # split by 256-col halves so the first output-projection
                # s-blocks can start before the full chunk is normalized
                for q in range(2):
                    qs = q * (SQ_CHUNK // 2)
                    nc.vector.tensor_mul(
                        out=oT_sb[hs:hs + HD, b,
                                  i * SQ_CHUNK + qs:
                                  i * SQ_CHUNK + qs + SQ_CHUNK // 2],
                        in0=po_h[h][0:HD, qs:qs + SQ_CHUNK // 2],
                        in1=rb[:, qs:qs + SQ_CHUNK // 2],
                    )

        def flush_proj(b, i, norm_po=None, rbs=None, tail=False):
            # generator; when norm_po is given, the normalization of each
            # 256-col half is emitted just before the output-projection
            # matmuls that consume it, shrinking the serial chunk-boundary
            # section
            if rbs is None:
                rbs = [None, None]
            for half in range(2):
                if norm_po is not None:
                    qs = half * (SQ_CHUNK // 2)
                    for h in range(HPC):
                        hs = h * HD
                        if half == 0:
                            rb = small.tile([HD, SQ_CHUNK], F32, tag="rb",
                                            name=f"rb{b}{i}{h}")
                            rbs[h] = rb
                            nc.vector.reciprocal(
                                out=rb, in_=norm_po[h][HD:P, :])
                        nc.vector.tensor_mul(
                            out=oT_sb[hs:hs + HD, b,
                                      i * SQ_CHUNK + qs:
                                      i * SQ_CHUNK + qs + SQ_CHUNK // 2],
                            in0=norm_po[h][0:HD, qs:qs + SQ_CHUNK // 2],
                            in1=rbs[h][:, qs:qs + SQ_CHUNK // 2],
                        )
                ot = outsb.tile([P, 2, D], MM_DT, tag="ot",
                                name=f"ot{b}_{i}_{half}")
                for si in range(2):
                    s = 4 * i + 2 * half + si
                    for cc in range(2):
                        # post-attention (tail) flushes split both the PSUM
                        # pool (pps/mpsum) and the drain engine (DVE/ACT):
                        # the serial tail drain halves, and the next hwloop
                        # body's projections (gated on pps via DVE) and
                        # attention (gated on mpsum via ACT) both restart
                        # ~13us earlier. Inline flushes keep DVE-only (ACT
                        # is busy with the exp stream there).
                        odd = cc == 1
                        if tail and odd:
                            pw = mpsum.tile([P, HPC, SQ_CHUNK], F32,
                                            tag="ps", name=f"tp{b}_{s}_{cc}")
                            pp = pw[:, 0, :]
                        else:
                            pp = pps.tile([P, SQ_CHUNK], F32, tag="pps",
                                          name=f"pp{b}_{s}_{cc}")
                        nc.tensor.matmul(
                            pp,
                            lhsT=oT_sb[:, b, ts(s, P)],
                            rhs=owT_sb[:, ts(cc, SQ_CHUNK)],
                            start=True,
                            stop=True,
                        )
                        if odd:
                            nc.scalar.copy(
                                out=ot[:, si, ts(cc, SQ_CHUNK)], in_=pp,
                            )
                        else:
                            nc.vector.tensor_copy(
                                out=ot[:, si, ts(cc, SQ_CHUNK)], in_=pp,
                            )
                    yield
                # rows [s0, s0+1] of this batch as [128, 2, D]
                s0 = 4 * i + 2 * half
                nc.sync.dma_start(out=out_r[b, :, s0:s0 + 2, :], in_=ot)

        # chunk-interleaved schedule: the projection generator for the
        # next chunk is drained round-robin with the attention j-loop of
        # the previous chunk, so ready projection matmuls sit between
        # potentially-stalling QK ops in the PE queue
        chunks = [(b, cn) for b in range(B) for cn in range(NSQ)]
        groups = []
        for n, (b, cn) in enumerate(chunks):
            prev = chunks[n - 1] if n > 0 else None
            groups.append(((b, cn), prev))
        groups.append((None, chunks[-1]))

        deferred = []
        for gi, (pitem, aitem) in enumerate(groups):
            gp = ph1_chunk(*pitem) if pitem is not None else None
            if aitem is not None:
                b, i = aitem
                po_h = [
                    opsum.tile([P, SQ_CHUNK], F32, tag="po",
                               name=f"po{b}_{i}_{h}")
                    for h in range(HPC)
                ]
                ga = att_chunk(b, i, po_h)
            else:
                ga = None
            while gp is not None or ga is not None:
                if gp is not None:
                    try:
                        next(gp)
                    except StopIteration:
                        gp = None
                if ga is not None:
                    try:
                        next(ga)
                    except StopIteration:
                        ga = None
            if aitem is not None:
                # defer the last chunks' output projections so they can
                # fill the exp-paced tail of the final attention chunk
                if gi >= len(groups) - 7:
                    flush_norm(b, i, po_h)
                    deferred.append((b, i))
                else:
                    for _ in flush_proj(b, i, norm_po=po_h,
                                        rbs=[None, None]):
                        pass
        for b, i in deferred:
            for _ in flush_proj(b, i, tail=True):
                pass


def get_module(repeat=1, hwloop=False, unroll=1):
    key = ("nc", repeat, hwloop, unroll)
    if key not in _module_cache:
        m = _build_module(repeat=repeat, hwloop=hwloop, unroll=unroll)
        m.compile()
        _module_cache[key] = m
    return _module_cache[key]


def make_in_maps(x, qw, qb, kw, kb, vw, vb, ow):
    mmdt = np.dtype(np.float16)
    f8dt = np.dtype(mybir.dt.np(mybir.dt.float8e4))
    xT = np.ascontiguousarray(x.transpose(0, 2, 1)).astype(mmdt)  # [B, D, S]
    xT8 = np.ascontiguousarray(
        x.transpose(0, 2, 1).astype(np.float32) * X8_SCALE).astype(f8dt)
    in_maps = []
    for c in range(NCORES):
        sl = slice(c * FPC, (c + 1) * FPC)
        m = {
            "xT": xT,
            "xT8": xT8,
            "qwT8": np.ascontiguousarray(
                qw[sl, :].T.astype(np.float32) * W8_SCALE).astype(f8dt),
            "kwT8": np.ascontiguousarray(
                kw[sl, :].T.astype(np.float32) * W8_SCALE).astype(f8dt),
            "vwT": np.ascontiguousarray(vw[sl, :].T).astype(mmdt),
            "qb": np.ascontiguousarray(qb[sl].reshape(FPC, 1)).astype(np.float32),
            "kb": np.ascontiguousarray(kb[sl].reshape(FPC, 1)).astype(np.float32),
            "vb": np.ascontiguousarray(vb[sl].reshape(FPC, 1)).astype(np.float32),
            "owT": np.ascontiguousarray(ow[:, sl].T).astype(mmdt),
        }
        in_maps.append(m)
    return in_maps


def kernel(x, qw, qb, kw, kb, vw, vb, ow, ob, _trace=False):
    x = np.asarray(x, dtype=np.float32)
    qw = np.asarray(qw, dtype=np.float32)
    qb = np.asarray(qb, dtype=np.float32)
    kw = np.asarray(kw, dtype=np.float32)
    kb = np.asarray(kb, dtype=np.float32)
    vw = np.asarray(vw, dtype=np.float32)
    vb = np.asarray(vb, dtype=np.float32)
    ow = np.asarray(ow, dtype=np.float32)
    ob = np.asarray(ob, dtype=np.float32)

    nc = get_module()
    in_maps = make_in_maps(x, qw, qb, kw, kb, vw, vb, ow)
    res = run_bass_kernel_spmd(
        nc, in_maps, core_ids=list(range(NCORES)), trace=_trace
    )
    acc = np.zeros((B, S, D), dtype=np.float64)
    for r in res.results:
        acc += r["out"].astype(np.float64)
    out = (acc + ob.astype(np.float64)).astype(np.float32)
    if _trace:
        kernel.last_results = res
    return out


# revision 34
# speedup vs baseline: 1.0774x; 1.0007x over previous
"""Head-sharded (tensor-parallel) causal attention block for 8 NeuronCores.

Model: B=2, S=2048, D=1024, H=16 heads (HD=64). Each core owns 2 heads
(128 features) of the QKV projections and attention, computes a partial
output projection (o_shard @ ow_shard), and the host sums the 8 partials
and adds the output bias.

Layout (single PSUM scope, chunk-interleaved pipeline):
  - Q/K projections run in fp8e4 DoubleRow mode (2 k-tiles of 128 per
    pass -> 4 passes instead of 8, 2x PE throughput). Host supplies
    x*16 and qw*256/kw*256 in fp8; the PSUM result carries a 4096x
    scale removed by the ACT-side drain (Identity activation with
    scale=1/4096 and the bias vector as its bias operand). V
    projection stays fp16 (v errors hit the output directly).
  - Projection and attention work interleave at j-tile granularity via
    generators: ready projection matmuls sit between potentially
    exp-stalled QK ops in the PE queue, so ScalarE's exp stream (the
    attention-phase pacer) overlaps projection matmuls.
  - V is projected into vT [feat, seq] (fp16), then moved to the PV
    lhsT layout V_aug[t, feat] via DMA XBAR transposes (no PE/PSUM).
  - V_aug columns 64:128 hold 1.0: the PV matmul emits the softmax
    denominator replicated on partitions 64:128, so normalization is a
    plain reciprocal + multiply (no partition broadcast).
  - The attention j-loop is software-pipelined: QK+exp for j are
    emitted one stage ahead of PV(j-1), so the in-order PE queue never
    parks on a PV waiting for its exp -- filler matmuls slot in behind
    the next QK instead. Causal mask: post-exp multiply of the diagonal
    128-block by a static 0/1 mask (DVE), emitted a full stage before
    PV consumes it. Fully-masked columns are skipped via col0 = 128*k.
  - Output projection partial[sq,1024] = oT.T @ owT in 512-wide halves
    through the projection PSUM pool; PSUM->SBUF copies ride DVE (ACT
    must stay free for the exp stream; GPSIMD cannot touch PSUM). The
    chunk's normalization (reciprocal of the PV-emitted denominator +
    multiply) is interleaved per 256-col half with the output
    projection that consumes it. Inline flush copies alternate DVE/ACT
    (ACT's exp queue is drained at chunk boundaries).
  - Most chunks' output projections are deferred past the final
    attention chunk; the deferred tail alternates both its PSUM pool
    (pps/mpsum, both idle there) and its drain engine (DVE/ACT), so
    the serial tail drain halves and the next hwloop body's
    projections (gated on pps via DVE) and attention (gated on mpsum
    via ACT) restart earlier.
  - PSUM budget (8 banks): proj/outproj 2x1, scores 2x2, PV accum 2x1.

Measured on the 8-core axon pod: ~192 us/body (baseline 219 us), rel
err 1.58e-2 vs the fp32 reference (gate 2e-2; the error is dominated
by the deliberate fp8 Q/K projections, measured identical in numpy
emulation).
"""

import numpy as np

import concourse.bass as bass
import concourse.mybir as mybir
import concourse.tile as tile
from concourse import bacc
from concourse.bass import ts
from concourse.bass_utils import run_bass_kernel_spmd

B, S, D, H = 2, 2048, 1024, 16
HD = D // H            # 64 head dim
NCORES = 8
FPC = D // NCORES      # 128 features per core
HPC = FPC // HD        # 2 heads per core
P = 128
SQ_CHUNK = 512         # query chunk (matmul free dim)
NSQ = S // SQ_CHUNK    # 4
NTB = S // P           # 16 t-blocks
DBLK = D // P          # 8 contraction blocks for fp16 projections
DBLK2 = DBLK // 2      # 4 DoubleRow passes for fp8 projections

F32 = mybir.dt.float32
MM_DT = mybir.dt.float16
F8 = mybir.dt.float8e4
X8_SCALE = 16.0        # x -> fp8 scale
W8_SCALE = 256.0       # qw/kw -> fp8 scale
DESCALE = 1.0 / (X8_SCALE * W8_SCALE)

_module_cache = {}


def _build_module(repeat=1, hwloop=False, unroll=1):
    nc = bacc.Bacc("TRN2", target_bir_lowering=False, debug=False)

    xT_d = nc.dram_tensor("xT", [B, D, S], MM_DT, kind="ExternalInput").ap()
    xT8_d = nc.dram_tensor("xT8", [B, D, S], F8, kind="ExternalInput").ap()
    qwT8_d = nc.dram_tensor("qwT8", [D, FPC], F8, kind="ExternalInput").ap()
    kwT8_d = nc.dram_tensor("kwT8", [D, FPC], F8, kind="ExternalInput").ap()
    vwT_d = nc.dram_tensor("vwT", [D, FPC], MM_DT, kind="ExternalInput").ap()
    qb_d = nc.dram_tensor("qb", [FPC, 1], F32, kind="ExternalInput").ap()
    kb_d = nc.dram_tensor("kb", [FPC, 1], F32, kind="ExternalInput").ap()
    vb_d = nc.dram_tensor("vb", [FPC, 1], F32, kind="ExternalInput").ap()
    owT_d = nc.dram_tensor("owT", [FPC, D], MM_DT, kind="ExternalInput").ap()
    out_d = nc.dram_tensor("out", [B, S, D], MM_DT, kind="ExternalOutput").ap()

    # [B, D, S] with D split into 8 blocks of 128 partitions
    xT_r = xT_d.rearrange("b (o p) s -> b p o s", p=P)
    # fp8 x in DoubleRow pair layout: d = 256*o2 + 128*two + p
    xT8_r = xT8_d.rearrange("b (o2 two p) s -> b p o2 two s", two=2, p=P)
    # [B, S, D] with S split into 128-row blocks (partition-first)
    out_r = out_d.rearrange("b (o p) d -> b p o d", p=P)

    with tile.TileContext(nc) as tc:
        with (
            tc.tile_pool(name="singles", bufs=1) as singles,
            tc.tile_pool(name="xin", bufs=3) as xin,
            tc.tile_pool(name="x8in", bufs=3) as x8in,
            tc.tile_pool(name="ptile", bufs=5) as ptile,
            tc.tile_pool(name="small", bufs=4) as small,
            tc.tile_pool(name="outsb", bufs=3) as outsb,
        ):
            # --- constants / persistent tensors (loaded once) ---
            qw8_sb = singles.tile([P, DBLK2, 2, FPC], F8, tag="qw8")
            kw8_sb = singles.tile([P, DBLK2, 2, FPC], F8, tag="kw8")
            vwT_sb = singles.tile([P, DBLK, FPC], MM_DT, tag="vw")
            nc.sync.dma_start(
                out=qw8_sb,
                in_=qwT8_d.rearrange("(o2 two p) m -> p o2 two m", two=2, p=P))
            nc.sync.dma_start(
                out=kw8_sb,
                in_=kwT8_d.rearrange("(o2 two p) m -> p o2 two m", two=2, p=P))
            nc.sync.dma_start(out=vwT_sb, in_=vwT_d.rearrange("(o p) m -> p o m", p=P))
            qb_sb = singles.tile([FPC, 1], F32, tag="qb")
            kb_sb = singles.tile([FPC, 1], F32, tag="kb")
            vb_sb = singles.tile([FPC, 1], F32, tag="vb")
            nc.sync.dma_start(out=qb_sb, in_=qb_d)
            nc.sync.dma_start(out=kb_sb, in_=kb_d)
            nc.sync.dma_start(out=vb_sb, in_=vb_d)
            owT_sb = singles.tile([FPC, D], MM_DT, tag="ow")
            nc.sync.dma_start(out=owT_sb, in_=owT_d)

            qT_sb = singles.tile([P, B, S], MM_DT, tag="qT")
            kT_sb = singles.tile([P, B, S], MM_DT, tag="kT")
            vT_sb = singles.tile([P, B, S], MM_DT, tag="vT")
            oT_sb = singles.tile([P, B, S], MM_DT, tag="oT")
            # V_aug[t, b, h, tblk, 0:64] = v features (fp16, written
            # directly by the DMA XBAR transposes); [.., 64:128] = 1.0 ->
            # the PV matmul emits the softmax denominator replicated on
            # partitions 64:128
            v_aug = singles.tile([P, B, HPC, NTB, P], MM_DT, tag="vaug")
            ones_sb = singles.tile([P, 1], F32, tag="ones")
            nc.vector.memset(ones_sb, 1.0)
            nc.vector.tensor_copy(
                out=v_aug[:, :, :, :, HD:P],
                in_=ones_sb[:, 0][:, None, None, None, None].to_broadcast(
                    [P, B, HPC, NTB, HD]),
            )
            # static causal 0/1 mask (fp16) for the post-exp multiply:
            # m128[t, c] = (c >= t)
            m128 = singles.tile([P, P], MM_DT, tag="m128")
            nc.gpsimd.memset(m128, 1.0)
            nc.gpsimd.affine_select(
                out=m128, in_=m128, compare_op=mybir.AluOpType.is_ge,
                fill=0.0, base=0, pattern=[[1, P]], channel_multiplier=-1,
            )

            # PSUM pools span all repetitions so consecutive bodies
            # pipeline through slot rotation instead of draining at each
            # body boundary
            with (
                tc.tile_pool(name="pps", bufs=2, space="PSUM") as pps,
                tc.tile_pool(name="mpsum", bufs=2, space="PSUM") as mpsum,
                tc.tile_pool(name="opsum", bufs=2, space="PSUM") as opsum,
            ):
                # ------ repetitions (>1 only for HW timing calibration) --
                if hwloop and repeat > 1:
                    with tc.For_i(0, repeat) as _i:
                        for _u in range(unroll):
                            _emit_body(nc, tc, locals())
                else:
                    for _rep in range(repeat):
                        _emit_body(nc, tc, locals())

    return nc


def _emit_body(nc, tc, env):
    g = type("G", (), env)
    singles, xin, x8in, ptile, small, outsb = (
        g.singles, g.xin, g.x8in, g.ptile, g.small, g.outsb)
    qw8_sb, kw8_sb, vwT_sb = g.qw8_sb, g.kw8_sb, g.vwT_sb
    qb_sb, kb_sb, vb_sb, owT_sb = g.qb_sb, g.kb_sb, g.vb_sb, g.owT_sb
    qT_sb, kT_sb, vT_sb, oT_sb, v_aug = g.qT_sb, g.kT_sb, g.vT_sb, g.oT_sb, g.v_aug
    m128 = g.m128
    xT_r, xT8_r, out_r = g.xT_r, g.xT8_r, g.out_r
    pps, mpsum, opsum = g.pps, g.mpsum, g.opsum

    if True:
        def ph1_chunk(b, cn):
            # generator: yields between matmul groups so the emitter can
            # interleave projection work into the attention j-loop.
            xt8 = x8in.tile([P, DBLK2, 2, SQ_CHUNK], F8, tag="xt8",
                            name=f"xt8_{b}{cn}")
            for g2 in range(2):
                nc.sync.dma_start(
                    out=xt8[:, 2 * g2:2 * g2 + 2],
                    in_=xT8_r[b, :, 2 * g2:2 * g2 + 2, :, ts(cn, SQ_CHUNK)])
            xt = xin.tile([P, DBLK, SQ_CHUNK], MM_DT, tag="xt",
                          name=f"xt{b}{cn}")
            for qd in range(4):
                nc.sync.dma_start(
                    out=xt[:, 2 * qd:2 * qd + 2, :],
                    in_=xT_r[b, :, 2 * qd:2 * qd + 2, ts(cn, SQ_CHUNK)])
            # Q/K in fp8 DoubleRow (4 passes of 2x128 contraction)
            for w8_sb, bias_sb, dst in (
                (qw8_sb, qb_sb, qT_sb),
                (kw8_sb, kb_sb, kT_sb),
            ):
                ps = pps.tile([P, SQ_CHUNK], F32, tag="pps",
                              name=f"prj{b}{cn}{id(dst)%97}")
                for o2 in range(DBLK2):
                    nc.tensor.matmul(
                        ps,
                        lhsT=w8_sb[:, o2],
                        rhs=xt8[:, o2],
                        perf_mode=mybir.MatmulPerfMode.DoubleRow,
                        start=(o2 == 0),
                        stop=(o2 == DBLK2 - 1),
                    )
                    if o2 == 1:
                        yield
                # fused descale (1/4096) + bias add, fp32 PSUM -> fp16
                # SBUF, on ACT (out = Copy(in*scale + bias)) to unload DVE
                nc.scalar.activation(
                    out=dst[:, b, ts(cn, SQ_CHUNK)], in_=ps,
                    func=mybir.ActivationFunctionType.Identity,
                    scale=DESCALE, bias=bias_sb,
                )
                yield
            # V projection in fp16 (8 passes)
            ps = pps.tile([P, SQ_CHUNK], F32, tag="pps",
                          name=f"prjv{b}{cn}")
            for o in range(DBLK):
                nc.tensor.matmul(
                    ps,
                    lhsT=vwT_sb[:, o, :],
                    rhs=xt[:, o, :],
                    start=(o == 0),
                    stop=(o == DBLK - 1),
                )
                if o % 4 == 3:
                    yield
            nc.vector.tensor_scalar_add(
                out=vT_sb[:, b, ts(cn, SQ_CHUNK)], in0=ps, scalar1=vb_sb,
            )
            yield
            # move V of this chunk into PV-lhsT layout via DMA XBAR
            # transpose: [64 feat, 512 seq] -> [128 t x 4 blocks, 64 feat]
            for h in range(HPC):
                hs = h * HD
                nc.sync.dma_start(
                    out=v_aug[:, b, h, 4 * cn:4 * cn + 4, 0:HD],
                    in_=vT_sb[hs:hs + HD, b, ts(cn, SQ_CHUNK)],
                    transpose=True,
                )

        def att_chunk(b, i, po_h):
            # software-pipelined j-loop: QK+exp for j run one stage ahead
            # of PV(j-1), so the PE's in-order queue never parks on a PV
            # that waits for its exp — filler matmuls (projections, output
            # projections) slot in behind QK(j+1) instead.
            jmax = 4 * i + 3
            pend = None
            for j in range(jmax + 2):
                if j <= jmax:
                    # columns < 128k of diagonal blocks are fully masked;
                    # skip them in QK, exp and PV
                    k = j - 4 * i
                    col0 = P * k if k > 0 else 0
                    ps = mpsum.tile([P, HPC, SQ_CHUNK], F32, tag="ps",
                                    name=f"ps{b}{i}{j}")
                    # two heads' QK in adjacent PE row-tiles
                    for h in range(HPC):
                        hs = h * HD
                        nc.tensor.matmul(
                            ps[:, h, col0:],
                            lhsT=kT_sb[hs:hs + HD, b, ts(j, P)],
                            rhs=qT_sb[hs:hs + HD, b,
                                      i * SQ_CHUNK + col0:(i + 1) * SQ_CHUNK],
                            start=True,
                            stop=True,
                        )
                    pt = ptile.tile([P, HPC, SQ_CHUNK], MM_DT, tag="pt",
                                    name=f"pt{b}{i}{j}")
                    nc.scalar.activation(
                        out=pt[:, :, col0:], in_=ps[:, :, col0:],
                        func=mybir.ActivationFunctionType.Exp,
                        scale=0.125,
                    )
                    if k >= 0:
                        # causal zero-fill post-exp; lands a full stage
                        # before PV consumes the diagonal block
                        nc.vector.tensor_tensor(
                            out=pt[:, :, col0:col0 + P],
                            in0=pt[:, :, col0:col0 + P],
                            in1=m128[:, None, :].to_broadcast([P, HPC, P]),
                            op=mybir.AluOpType.mult,
                        )
                    cur = (j, col0, pt)
                else:
                    cur = None
                if pend is not None:
                    pj, pcol0, ppt = pend
                    for h in range(HPC):
                        nc.tensor.matmul(
                            po_h[h][:, pcol0:],
                            lhsT=v_aug[:, b, h, pj, :],
                            rhs=ppt[:, h, pcol0:],
                            start=(pj == 0),
                            stop=(pj == jmax),
                            skip_group_check=True,
                        )
                pend = cur
                yield

        def flush_norm(b, i, po_h):
            for h in range(HPC):
                hs = h * HD
                rb = small.tile([HD, SQ_CHUNK], F32, tag="rb",
                                name=f"rb{b}{i}{h}")
                nc.vector.reciprocal(out=rb, in_=po_h[h][HD:P, :])
                # split by 256-col halves so the first output-projection
                # s-blocks can start before the full chunk is normalized
                for q in range(2):
                    qs = q * (SQ_CHUNK // 2)
                    nc.vector.tensor_mul(
                        out=oT_sb[hs:hs + HD, b,
                                  i * SQ_CHUNK + qs:
                                  i * SQ_CHUNK + qs + SQ_CHUNK // 2],
                        in0=po_h[h][0:HD, qs:qs + SQ_CHUNK // 2],
                        in1=rb[:, qs:qs + SQ_CHUNK // 2],
                    )

        def flush_proj(b, i, norm_po=None, rbs=None, tail=False):
            # generator; when norm_po is given, the normalization of each
            # 256-col half is emitted just before the output-projection
            # matmuls that consume it, shrinking the serial chunk-boundary
            # section
            if rbs is None:
                rbs = [None, None]
            for half in range(2):
                if norm_po is not None:
                    qs = half * (SQ_CHUNK // 2)
                    for h in range(HPC):
                        hs = h * HD
                        if half == 0:
                            rb = small.tile([HD, SQ_CHUNK], F32, tag="rb",
                                            name=f"rb{b}{i}{h}")
                            rbs[h] = rb
                            nc.vector.reciprocal(
                                out=rb, in_=norm_po[h][HD:P, :])
                        nc.vector.tensor_mul(
                            out=oT_sb[hs:hs + HD, b,
                                      i * SQ_CHUNK + qs:
                                      i * SQ_CHUNK + qs + SQ_CHUNK // 2],
                            in0=norm_po[h][0:HD, qs:qs + SQ_CHUNK // 2],
                            in1=rbs[h][:, qs:qs + SQ_CHUNK // 2],
                        )
                ot = outsb.tile([P, 2, D], MM_DT, tag="ot",
                                name=f"ot{b}_{i}_{half}")
                for si in range(2):
                    s = 4 * i + 2 * half + si
                    for cc in range(2):
                        # post-attention (tail) flushes split both the PSUM
                        # pool (pps/mpsum) and the drain engine (DVE/ACT):
                        # the serial tail drain halves, and the next hwloop
                        # body's projections (gated on pps via DVE) and
                        # attention (gated on mpsum via ACT) both restart
                        # ~13us earlier. Inline flushes keep DVE-only (ACT
                        # is busy with the exp stream there).
                        odd = cc == 1
                        if tail and odd:
                            pw = mpsum.tile([P, HPC, SQ_CHUNK], F32,
                                            tag="ps", name=f"tp{b}_{s}_{cc}")
                            pp = pw[:, 0, :]
                        else:
                            pp = pps.tile([P, SQ_CHUNK], F32, tag="pps",
                                          name=f"pp{b}_{s}_{cc}")
                        nc.tensor.matmul(
                            pp,
                            lhsT=oT_sb[:, b, ts(s, P)],
                            rhs=owT_sb[:, ts(cc, SQ_CHUNK)],
                            start=True,
                            stop=True,
                        )
                        if odd:
                            nc.scalar.copy(
                                out=ot[:, si, ts(cc, SQ_CHUNK)], in_=pp,
                            )
                        else:
                            nc.vector.tensor_copy(
                                out=ot[:, si, ts(cc, SQ_CHUNK)], in_=pp,
                            )
                    yield
                # rows [s0, s0+1] of this batch as [128, 2, D]
                s0 = 4 * i + 2 * half
                nc.sync.dma_start(out=out_r[b, :, s0:s0 + 2, :], in_=ot)

        # chunk-interleaved schedule: the projection generator for the
        # next chunk is drained round-robin with the attention j-loop of
        # the previous chunk, so ready projection matmuls sit between
        # potentially-stalling QK ops in the PE queue
        chunks = [(b, cn) for b in range(B) for cn in range(NSQ)]
        groups = []
        for n, (b, cn) in enumerate(chunks):
            prev = chunks[n - 1] if n > 0 else None
            groups.append(((b, cn), prev))
        groups.append((None, chunks[-1]))

        deferred = []
        for gi, (pitem, aitem) in enumerate(groups):
            gp = ph1_chunk(*pitem) if pitem is not None else None
            if aitem is not None:
                b, i = aitem
                po_h = [
                    opsum.tile([P, SQ_CHUNK], F32, tag="po",
                               name=f"po{b}_{i}_{h}")
                    for h in range(HPC)
                ]
                ga = att_chunk(b, i, po_h)
            else:
                ga = None
            while gp is not None or ga is not None:
                if gp is not None:
                    try:
                        next(gp)
                    except StopIteration:
                        gp = None
                if ga is not None:
                    try:
                        next(ga)
                    except StopIteration:
                        ga = None
            if aitem is not None:
                # defer the last chunks' output projections so they can
                # fill the exp-paced tail of the final attention chunk
                if gi >= len(groups) - 9:
                    flush_norm(b, i, po_h)
                    deferred.append((b, i))
                else:
                    for _ in flush_proj(b, i, norm_po=po_h,
                                        rbs=[None, None]):
                        pass
        for b, i in deferred:
            for _ in flush_proj(b, i, tail=True):
                pass


def get_module(repeat=1, hwloop=False, unroll=1):
    key = ("nc", repeat, hwloop, unroll)
    if key not in _module_cache:
        m = _build_module(repeat=repeat, hwloop=hwloop, unroll=unroll)
        m.compile()
        _module_cache[key] = m
    return _module_cache[key]


def make_in_maps(x, qw, qb, kw, kb, vw, vb, ow):
    mmdt = np.dtype(np.float16)
    f8dt = np.dtype(mybir.dt.np(mybir.dt.float8e4))
    xT = np.ascontiguousarray(x.transpose(0, 2, 1)).astype(mmdt)  # [B, D, S]
    xT8 = np.ascontiguousarray(
        x.transpose(0, 2, 1).astype(np.float32) * X8_SCALE).astype(f8dt)
    in_maps = []
    for c in range(NCORES):
        sl = slice(c * FPC, (c + 1) * FPC)
        m = {
            "xT": xT,
            "xT8": xT8,
            "qwT8": np.ascontiguousarray(
                qw[sl, :].T.astype(np.float32) * W8_SCALE).astype(f8dt),
            "kwT8": np.ascontiguousarray(
                kw[sl, :].T.astype(np.float32) * W8_SCALE).astype(f8dt),
            "vwT": np.ascontiguousarray(vw[sl, :].T).astype(mmdt),
            "qb": np.ascontiguousarray(qb[sl].reshape(FPC, 1)).astype(np.float32),
            "kb": np.ascontiguousarray(kb[sl].reshape(FPC, 1)).astype(np.float32),
            "vb": np.ascontiguousarray(vb[sl].reshape(FPC, 1)).astype(np.float32),
            "owT": np.ascontiguousarray(ow[:, sl].T).astype(mmdt),
        }
        in_maps.append(m)
    return in_maps


def kernel(x, qw, qb, kw, kb, vw, vb, ow, ob, _trace=False):
    x = np.asarray(x, dtype=np.float32)
    qw = np.asarray(qw, dtype=np.float32)
    qb = np.asarray(qb, dtype=np.float32)
    kw = np.asarray(kw, dtype=np.float32)
    kb = np.asarray(kb, dtype=np.float32)
    vw = np.asarray(vw, dtype=np.float32)
    vb = np.asarray(vb, dtype=np.float32)
    ow = np.asarray(ow, dtype=np.float32)
    ob = np.asarray(ob, dtype=np.float32)

    nc = get_module()
    in_maps = make_in_maps(x, qw, qb, kw, kb, vw, vb, ow)
    res = run_bass_kernel_spmd(
        nc, in_maps, core_ids=list(range(NCORES)), trace=_trace
    )
    acc = np.zeros((B, S, D), dtype=np.float64)
    for r in res.results:
        acc += r["out"].astype(np.float64)
    out = (acc + ob.astype(np.float64)).astype(np.float32)
    if _trace:
        kernel.last_results = res
    return out


# revision 35
# speedup vs baseline: 1.1195x; 1.0390x over previous
"""Head-sharded (tensor-parallel) causal attention block for 8 NeuronCores.

Model: B=2, S=2048, D=1024, H=16 heads (HD=64). Each core owns 2 heads
(128 features) of the QKV projections and attention, computes a partial
output projection (o_shard @ ow_shard), and the host sums the 8 partials
and adds the output bias.

Layout (single PSUM scope, chunk-interleaved pipeline):
  - Q/K projections run in fp8e4 DoubleRow mode (2 k-tiles of 128 per
    pass -> 4 passes instead of 8, 2x PE throughput). Host supplies
    x*16 and qw*256/kw*256 in fp8; the PSUM result carries a 4096x
    scale removed by the ACT-side drain (Identity activation with
    scale=1/4096 and the bias vector as its bias operand). V
    projection stays fp16 (v errors hit the output directly).
  - Projection and attention work interleave at j-tile granularity via
    generators: ready projection matmuls sit between potentially
    exp-stalled QK ops in the PE queue, so ScalarE's exp stream (the
    attention-phase pacer) overlaps projection matmuls.
  - V is projected into vT [feat, seq] (fp16), then moved to the PV
    lhsT layout V_aug[t, feat] via DMA XBAR transposes (no PE/PSUM).
  - V_aug columns 64:128 hold 1.0: the PV matmul emits the softmax
    denominator replicated on partitions 64:128, so normalization is a
    plain reciprocal + multiply (no partition broadcast).
  - The attention j-loop is software-pipelined: QK+exp for j are
    emitted one stage ahead of PV(j-1), so the in-order PE queue never
    parks on a PV waiting for its exp -- filler matmuls slot in behind
    the next QK instead. Causal mask: post-exp multiply of the diagonal
    128-block by a static 0/1 mask (DVE), emitted a full stage before
    PV consumes it. Fully-masked columns are skipped via col0 = 128*k.
  - Output projection partial[sq,1024] = oT.T @ owT in 512-wide halves
    through the projection PSUM pool; PSUM->SBUF copies ride DVE (ACT
    must stay free for the exp stream; GPSIMD cannot touch PSUM). The
    chunk's normalization (reciprocal of the PV-emitted denominator +
    multiply) is interleaved per 256-col half with the output
    projection that consumes it. Inline flush copies alternate DVE/ACT
    (ACT's exp queue is drained at chunk boundaries).
  - Most chunks' output projections are deferred past the final
    attention chunk; the deferred tail alternates both its PSUM pool
    (pps/mpsum, both idle there) and its drain engine (DVE/ACT), so
    the serial tail drain halves and the next hwloop body's
    projections (gated on pps via DVE) and attention (gated on mpsum
    via ACT) restart earlier.
  - PSUM budget (8 banks): proj/outproj 2x1, scores 2x2, PV accum 2x1.

Measured on the 8-core axon pod: ~192 us/body (baseline 219 us), rel
err 1.58e-2 vs the fp32 reference (gate 2e-2; the error is dominated
by the deliberate fp8 Q/K projections, measured identical in numpy
emulation).
"""

import numpy as np

import concourse.bass as bass
import concourse.mybir as mybir
import concourse.tile as tile
from concourse import bacc
from concourse.bass import ts
from concourse.bass_utils import run_bass_kernel_spmd

B, S, D, H = 2, 2048, 1024, 16
HD = D // H            # 64 head dim
NCORES = 8
FPC = D // NCORES      # 128 features per core
HPC = FPC // HD        # 2 heads per core
P = 128
SQ_CHUNK = 512         # query chunk (matmul free dim)
NSQ = S // SQ_CHUNK    # 4
NTB = S // P           # 16 t-blocks
DBLK = D // P          # 8 contraction blocks for fp16 projections
DBLK2 = DBLK // 2      # 4 DoubleRow passes for fp8 projections

F32 = mybir.dt.float32
MM_DT = mybir.dt.float16
F8 = mybir.dt.float8e4
X8_SCALE = 16.0        # x -> fp8 scale
W8_SCALE = 256.0       # qw/kw -> fp8 scale
DESCALE = 1.0 / (X8_SCALE * W8_SCALE)

_module_cache = {}


def _build_module(repeat=1, hwloop=False, unroll=1):
    nc = bacc.Bacc("TRN2", target_bir_lowering=False, debug=False)

    xT_d = nc.dram_tensor("xT", [B, D, S], MM_DT, kind="ExternalInput").ap()
    xT8_d = nc.dram_tensor("xT8", [B, D, S], F8, kind="ExternalInput").ap()
    qwT8_d = nc.dram_tensor("qwT8", [D, FPC], F8, kind="ExternalInput").ap()
    kwT8_d = nc.dram_tensor("kwT8", [D, FPC], F8, kind="ExternalInput").ap()
    vwT_d = nc.dram_tensor("vwT", [D, FPC], MM_DT, kind="ExternalInput").ap()
    qb_d = nc.dram_tensor("qb", [FPC, 1], F32, kind="ExternalInput").ap()
    kb_d = nc.dram_tensor("kb", [FPC, 1], F32, kind="ExternalInput").ap()
    vb_d = nc.dram_tensor("vb", [FPC, 1], F32, kind="ExternalInput").ap()
    owT_d = nc.dram_tensor("owT", [FPC, D], MM_DT, kind="ExternalInput").ap()
    out_d = nc.dram_tensor("out", [B, S, D], MM_DT, kind="ExternalOutput").ap()

    # [B, D, S] with D split into 8 blocks of 128 partitions
    xT_r = xT_d.rearrange("b (o p) s -> b p o s", p=P)
    # fp8 x in DoubleRow pair layout: d = 256*o2 + 128*two + p
    xT8_r = xT8_d.rearrange("b (o2 two p) s -> b p o2 two s", two=2, p=P)
    # [B, S, D] with S split into 128-row blocks (partition-first)
    out_r = out_d.rearrange("b (o p) d -> b p o d", p=P)

    with tile.TileContext(nc) as tc:
        with (
            tc.tile_pool(name="singles", bufs=1) as singles,
            tc.tile_pool(name="xin", bufs=4) as xin,
            tc.tile_pool(name="x8in", bufs=4) as x8in,
            tc.tile_pool(name="ptile", bufs=6) as ptile,
            tc.tile_pool(name="small", bufs=4) as small,
            tc.tile_pool(name="outsb", bufs=4) as outsb,
        ):
            # --- constants / persistent tensors (loaded once) ---
            qw8_sb = singles.tile([P, DBLK2, 2, FPC], F8, tag="qw8")
            kw8_sb = singles.tile([P, DBLK2, 2, FPC], F8, tag="kw8")
            vwT_sb = singles.tile([P, DBLK, FPC], MM_DT, tag="vw")
            nc.sync.dma_start(
                out=qw8_sb,
                in_=qwT8_d.rearrange("(o2 two p) m -> p o2 two m", two=2, p=P))
            nc.sync.dma_start(
                out=kw8_sb,
                in_=kwT8_d.rearrange("(o2 two p) m -> p o2 two m", two=2, p=P))
            nc.sync.dma_start(out=vwT_sb, in_=vwT_d.rearrange("(o p) m -> p o m", p=P))
            qb_sb = singles.tile([FPC, 1], F32, tag="qb")
            kb_sb = singles.tile([FPC, 1], F32, tag="kb")
            vb_sb = singles.tile([FPC, 1], F32, tag="vb")
            nc.sync.dma_start(out=qb_sb, in_=qb_d)
            nc.sync.dma_start(out=kb_sb, in_=kb_d)
            nc.sync.dma_start(out=vb_sb, in_=vb_d)
            owT_sb = singles.tile([FPC, D], MM_DT, tag="ow")
            nc.sync.dma_start(out=owT_sb, in_=owT_d)

            qT_sb = singles.tile([P, B, S], MM_DT, tag="qT")
            kT_sb = singles.tile([P, B, S], MM_DT, tag="kT")
            vT_sb = singles.tile([P, B, S], MM_DT, tag="vT")
            oT_sb = singles.tile([P, B, S], MM_DT, tag="oT")
            # V_aug[t, b, h, tblk, 0:64] = v features (fp16, written
            # directly by the DMA XBAR transposes); [.., 64:128] = 1.0 ->
            # the PV matmul emits the softmax denominator replicated on
            # partitions 64:128
            v_aug = singles.tile([P, B, HPC, NTB, P], MM_DT, tag="vaug")
            ones_sb = singles.tile([P, 1], F32, tag="ones")
            nc.vector.memset(ones_sb, 1.0)
            nc.vector.tensor_copy(
                out=v_aug[:, :, :, :, HD:P],
                in_=ones_sb[:, 0][:, None, None, None, None].to_broadcast(
                    [P, B, HPC, NTB, HD]),
            )
            # static causal 0/1 mask (fp16) for the post-exp multiply:
            # m128[t, c] = (c >= t)
            m128 = singles.tile([P, P], MM_DT, tag="m128")
            nc.gpsimd.memset(m128, 1.0)
            nc.gpsimd.affine_select(
                out=m128, in_=m128, compare_op=mybir.AluOpType.is_ge,
                fill=0.0, base=0, pattern=[[1, P]], channel_multiplier=-1,
            )

            # PSUM pools span all repetitions so consecutive bodies
            # pipeline through slot rotation instead of draining at each
            # body boundary
            with (
                tc.tile_pool(name="pps", bufs=2, space="PSUM") as pps,
                tc.tile_pool(name="mpsum", bufs=2, space="PSUM") as mpsum,
                tc.tile_pool(name="opsum", bufs=2, space="PSUM") as opsum,
            ):
                # ------ repetitions (>1 only for HW timing calibration) --
                if hwloop and repeat > 1:
                    with tc.For_i(0, repeat) as _i:
                        for _u in range(unroll):
                            _emit_body(nc, tc, locals())
                else:
                    for _rep in range(repeat):
                        _emit_body(nc, tc, locals())

    return nc


def _emit_body(nc, tc, env):
    g = type("G", (), env)
    singles, xin, x8in, ptile, small, outsb = (
        g.singles, g.xin, g.x8in, g.ptile, g.small, g.outsb)
    qw8_sb, kw8_sb, vwT_sb = g.qw8_sb, g.kw8_sb, g.vwT_sb
    qb_sb, kb_sb, vb_sb, owT_sb = g.qb_sb, g.kb_sb, g.vb_sb, g.owT_sb
    qT_sb, kT_sb, vT_sb, oT_sb, v_aug = g.qT_sb, g.kT_sb, g.vT_sb, g.oT_sb, g.v_aug
    m128 = g.m128
    xT_r, xT8_r, out_r = g.xT_r, g.xT8_r, g.out_r
    pps, mpsum, opsum = g.pps, g.mpsum, g.opsum

    if True:
        def ph1_chunk(b, cn):
            # generator: yields between matmul groups so the emitter can
            # interleave projection work into the attention j-loop.
            xt8 = x8in.tile([P, DBLK2, 2, SQ_CHUNK], F8, tag="xt8",
                            name=f"xt8_{b}{cn}")
            for g2 in range(2):
                nc.sync.dma_start(
                    out=xt8[:, 2 * g2:2 * g2 + 2],
                    in_=xT8_r[b, :, 2 * g2:2 * g2 + 2, :, ts(cn, SQ_CHUNK)])
            xt = xin.tile([P, DBLK, SQ_CHUNK], MM_DT, tag="xt",
                          name=f"xt{b}{cn}")
            for qd in range(4):
                nc.sync.dma_start(
                    out=xt[:, 2 * qd:2 * qd + 2, :],
                    in_=xT_r[b, :, 2 * qd:2 * qd + 2, ts(cn, SQ_CHUNK)])
            # Q/K in fp8 DoubleRow (4 passes of 2x128 contraction)
            for w8_sb, bias_sb, dst in (
                (qw8_sb, qb_sb, qT_sb),
                (kw8_sb, kb_sb, kT_sb),
            ):
                ps = pps.tile([P, SQ_CHUNK], F32, tag="pps",
                              name=f"prj{b}{cn}{id(dst)%97}")
                for o2 in range(DBLK2):
                    nc.tensor.matmul(
                        ps,
                        lhsT=w8_sb[:, o2],
                        rhs=xt8[:, o2],
                        perf_mode=mybir.MatmulPerfMode.DoubleRow,
                        start=(o2 == 0),
                        stop=(o2 == DBLK2 - 1),
                    )
                    if o2 == 1:
                        yield
                # fused descale (1/4096) + bias add, fp32 PSUM -> fp16
                # SBUF, on ACT (out = Copy(in*scale + bias)) to unload DVE
                nc.scalar.activation(
                    out=dst[:, b, ts(cn, SQ_CHUNK)], in_=ps,
                    func=mybir.ActivationFunctionType.Identity,
                    scale=DESCALE, bias=bias_sb,
                )
                yield
            # V projection in fp16 (8 passes)
            ps = pps.tile([P, SQ_CHUNK], F32, tag="pps",
                          name=f"prjv{b}{cn}")
            for o in range(DBLK):
                nc.tensor.matmul(
                    ps,
                    lhsT=vwT_sb[:, o, :],
                    rhs=xt[:, o, :],
                    start=(o == 0),
                    stop=(o == DBLK - 1),
                )
                if o % 4 == 3:
                    yield
            nc.vector.tensor_scalar_add(
                out=vT_sb[:, b, ts(cn, SQ_CHUNK)], in0=ps, scalar1=vb_sb,
            )
            yield
            # move V of this chunk into PV-lhsT layout via DMA XBAR
            # transpose: [64 feat, 512 seq] -> [128 t x 4 blocks, 64 feat]
            for h in range(HPC):
                hs = h * HD
                nc.sync.dma_start(
                    out=v_aug[:, b, h, 4 * cn:4 * cn + 4, 0:HD],
                    in_=vT_sb[hs:hs + HD, b, ts(cn, SQ_CHUNK)],
                    transpose=True,
                )

        def att_chunk(b, i, po_h):
            # software-pipelined j-loop: QK+exp for j run one stage ahead
            # of PV(j-1), so the PE's in-order queue never parks on a PV
            # that waits for its exp — filler matmuls (projections, output
            # projections) slot in behind QK(j+1) instead.
            jmax = 4 * i + 3
            pend = None
            for j in range(jmax + 2):
                if j <= jmax:
                    # columns < 128k of diagonal blocks are fully masked;
                    # skip them in QK, exp and PV
                    k = j - 4 * i
                    col0 = P * k if k > 0 else 0
                    ps = mpsum.tile([P, HPC, SQ_CHUNK], F32, tag="ps",
                                    name=f"ps{b}{i}{j}")
                    # two heads' QK in adjacent PE row-tiles
                    for h in range(HPC):
                        hs = h * HD
                        nc.tensor.matmul(
                            ps[:, h, col0:],
                            lhsT=kT_sb[hs:hs + HD, b, ts(j, P)],
                            rhs=qT_sb[hs:hs + HD, b,
                                      i * SQ_CHUNK + col0:(i + 1) * SQ_CHUNK],
                            start=True,
                            stop=True,
                        )
                    pt = ptile.tile([P, HPC, SQ_CHUNK], MM_DT, tag="pt",
                                    name=f"pt{b}{i}{j}")
                    nc.scalar.activation(
                        out=pt[:, :, col0:], in_=ps[:, :, col0:],
                        func=mybir.ActivationFunctionType.Exp,
                        scale=0.125,
                    )
                    if k >= 0:
                        # causal zero-fill post-exp; lands a full stage
                        # before PV consumes the diagonal block
                        nc.vector.tensor_tensor(
                            out=pt[:, :, col0:col0 + P],
                            in0=pt[:, :, col0:col0 + P],
                            in1=m128[:, None, :].to_broadcast([P, HPC, P]),
                            op=mybir.AluOpType.mult,
                        )
                    cur = (j, col0, pt)
                else:
                    cur = None
                if pend is not None:
                    pj, pcol0, ppt = pend
                    for h in range(HPC):
                        nc.tensor.matmul(
                            po_h[h][:, pcol0:],
                            lhsT=v_aug[:, b, h, pj, :],
                            rhs=ppt[:, h, pcol0:],
                            start=(pj == 0),
                            stop=(pj == jmax),
                            skip_group_check=True,
                        )
                pend = cur
                yield

        def flush_norm(b, i, po_h):
            for h in range(HPC):
                hs = h * HD
                rb = small.tile([HD, SQ_CHUNK], F32, tag="rb",
                                name=f"rb{b}{i}{h}")
                nc.vector.reciprocal(out=rb, in_=po_h[h][HD:P, :])
                # split by 256-col halves so the first output-projection
                # s-blocks can start before the full chunk is normalized
                for q in range(2):
                    qs = q * (SQ_CHUNK // 2)
                    nc.vector.tensor_mul(
                        out=oT_sb[hs:hs + HD, b,
                                  i * SQ_CHUNK + qs:
                                  i * SQ_CHUNK + qs + SQ_CHUNK // 2],
                        in0=po_h[h][0:HD, qs:qs + SQ_CHUNK // 2],
                        in1=rb[:, qs:qs + SQ_CHUNK // 2],
                    )

        def flush_proj(b, i, norm_po=None, rbs=None, tail=False):
            # generator; when norm_po is given, the normalization of each
            # 256-col half is emitted just before the output-projection
            # matmuls that consume it, shrinking the serial chunk-boundary
            # section
            if rbs is None:
                rbs = [None, None]
            for half in range(2):
                if norm_po is not None:
                    qs = half * (SQ_CHUNK // 2)
                    for h in range(HPC):
                        hs = h * HD
                        if half == 0:
                            rb = small.tile([HD, SQ_CHUNK], F32, tag="rb",
                                            name=f"rb{b}{i}{h}")
                            rbs[h] = rb
                            nc.vector.reciprocal(
                                out=rb, in_=norm_po[h][HD:P, :])
                        nc.vector.tensor_mul(
                            out=oT_sb[hs:hs + HD, b,
                                      i * SQ_CHUNK + qs:
                                      i * SQ_CHUNK + qs + SQ_CHUNK // 2],
                            in0=norm_po[h][0:HD, qs:qs + SQ_CHUNK // 2],
                            in1=rbs[h][:, qs:qs + SQ_CHUNK // 2],
                        )
                ot = outsb.tile([P, 2, D], MM_DT, tag="ot",
                                name=f"ot{b}_{i}_{half}")
                for si in range(2):
                    s = 4 * i + 2 * half + si
                    for cc in range(2):
                        # post-attention (tail) flushes split both the PSUM
                        # pool (pps/mpsum) and the drain engine (DVE/ACT):
                        # the serial tail drain halves, and the next hwloop
                        # body's projections (gated on pps via DVE) and
                        # attention (gated on mpsum via ACT) both restart
                        # ~13us earlier. Inline flushes keep DVE-only (ACT
                        # is busy with the exp stream there).
                        odd = cc == 1
                        if tail and odd:
                            pw = mpsum.tile([P, HPC, SQ_CHUNK], F32,
                                            tag="ps", name=f"tp{b}_{s}_{cc}")
                            pp = pw[:, 0, :]
                        else:
                            pp = pps.tile([P, SQ_CHUNK], F32, tag="pps",
                                          name=f"pp{b}_{s}_{cc}")
                        nc.tensor.matmul(
                            pp,
                            lhsT=oT_sb[:, b, ts(s, P)],
                            rhs=owT_sb[:, ts(cc, SQ_CHUNK)],
                            start=True,
                            stop=True,
                        )
                        if odd:
                            nc.scalar.copy(
                                out=ot[:, si, ts(cc, SQ_CHUNK)], in_=pp,
                            )
                        else:
                            nc.vector.tensor_copy(
                                out=ot[:, si, ts(cc, SQ_CHUNK)], in_=pp,
                            )
                    yield
                # rows [s0, s0+1] of this batch as [128, 2, D]
                s0 = 4 * i + 2 * half
                nc.sync.dma_start(out=out_r[b, :, s0:s0 + 2, :], in_=ot)

        # chunk-interleaved schedule: the projection generator for the
        # next chunk is drained round-robin with the attention j-loop of
        # the previous chunk, so ready projection matmuls sit between
        # potentially-stalling QK ops in the PE queue
        chunks = [(b, cn) for b in range(B) for cn in range(NSQ)]
        groups = []
        for n, (b, cn) in enumerate(chunks):
            prev = chunks[n - 1] if n > 0 else None
            groups.append(((b, cn), prev))
        groups.append((None, chunks[-1]))

        deferred = []
        for gi, (pitem, aitem) in enumerate(groups):
            gp = ph1_chunk(*pitem) if pitem is not None else None
            if aitem is not None:
                b, i = aitem
                po_h = [
                    opsum.tile([P, SQ_CHUNK], F32, tag="po",
                               name=f"po{b}_{i}_{h}")
                    for h in range(HPC)
                ]
                ga = att_chunk(b, i, po_h)
            else:
                ga = None
            while gp is not None or ga is not None:
                if gp is not None:
                    try:
                        next(gp)
                    except StopIteration:
                        gp = None
                if ga is not None:
                    try:
                        next(ga)
                    except StopIteration:
                        ga = None
            if aitem is not None:
                # defer the last chunks' output projections so they can
                # fill the exp-paced tail of the final attention chunk
                if gi >= len(groups) - 9:
                    flush_norm(b, i, po_h)
                    deferred.append((b, i))
                else:
                    for _ in flush_proj(b, i, norm_po=po_h,
                                        rbs=[None, None]):
                        pass
        for b, i in deferred:
            for _ in flush_proj(b, i, tail=True):
                pass


def get_module(repeat=1, hwloop=False, unroll=1):
    key = ("nc", repeat, hwloop, unroll)
    if key not in _module_cache:
        m = _build_module(repeat=repeat, hwloop=hwloop, unroll=unroll)
        m.compile()
        _module_cache[key] = m
    return _module_cache[key]


def make_in_maps(x, qw, qb, kw, kb, vw, vb, ow):
    mmdt = np.dtype(np.float16)
    f8dt = np.dtype(mybir.dt.np(mybir.dt.float8e4))
    xT = np.ascontiguousarray(x.transpose(0, 2, 1)).astype(mmdt)  # [B, D, S]
    xT8 = np.ascontiguousarray(
        x.transpose(0, 2, 1).astype(np.float32) * X8_SCALE).astype(f8dt)
    in_maps = []
    for c in range(NCORES):
        sl = slice(c * FPC, (c + 1) * FPC)
        m = {
            "xT": xT,
            "xT8": xT8,
            "qwT8": np.ascontiguousarray(
                qw[sl, :].T.astype(np.float32) * W8_SCALE).astype(f8dt),
            "kwT8": np.ascontiguousarray(
                kw[sl, :].T.astype(np.float32) * W8_SCALE).astype(f8dt),
            "vwT": np.ascontiguousarray(vw[sl, :].T).astype(mmdt),
            "qb": np.ascontiguousarray(qb[sl].reshape(FPC, 1)).astype(np.float32),
            "kb": np.ascontiguousarray(kb[sl].reshape(FPC, 1)).astype(np.float32),
            "vb": np.ascontiguousarray(vb[sl].reshape(FPC, 1)).astype(np.float32),
            "owT": np.ascontiguousarray(ow[:, sl].T).astype(mmdt),
        }
        in_maps.append(m)
    return in_maps


def kernel(x, qw, qb, kw, kb, vw, vb, ow, ob, _trace=False):
    x = np.asarray(x, dtype=np.float32)
    qw = np.asarray(qw, dtype=np.float32)
    qb = np.asarray(qb, dtype=np.float32)
    kw = np.asarray(kw, dtype=np.float32)
    kb = np.asarray(kb, dtype=np.float32)
    vw = np.asarray(vw, dtype=np.float32)
    vb = np.asarray(vb, dtype=np.float32)
    ow = np.asarray(ow, dtype=np.float32)
    ob = np.asarray(ob, dtype=np.float32)

    nc = get_module()
    in_maps = make_in_maps(x, qw, qb, kw, kb, vw, vb, ow)
    res = run_bass_kernel_spmd(
        nc, in_maps, core_ids=list(range(NCORES)), trace=_trace
    )
    acc = np.zeros((B, S, D), dtype=np.float64)
    for r in res.results:
        acc += r["out"].astype(np.float64)
    out = (acc + ob.astype(np.float64)).astype(np.float32)
    if _trace:
        kernel.last_results = res
    return out
